# revision 36
# baseline (speedup 1.0000x reference)
"""Batch whitening (Cholesky) kernel for Trainium2, 8 NeuronCores.

Computes, for X [32768, 1024] (matching the reference nn_BWCholeskyBlock):
    mean = X.mean(0); xc = X - mean; cov = xc.T @ xc / N
    L = chol(cov + eps I);  Y = (L^-1 xc^T).T + beta

Strategy (data-parallel over batch, 8 cores; harness gate rel_err < 2e-2):
  Phase 1 (device): per-core partial gram  G_i = Xq_i^T Xq_i  and column
     sums.  Two dtype modes:
       fp16:  [128,256] gram tiles                   (~72us MM stream)
       fp8dr: e4m3 DoubleRow [128,512] gram tiles, K=256 per matmul --
              runs at the fp8 PE peak               (~44us MM stream)
     Column sums ride on VectorE (wide [P,2,F] fp32 accumulators),
     fully hidden under the matmul stream.
  Host: reduce partials, mirror the triangle -> mean, cov; Cholesky +
     triangular inverse of the small [F,F] factor (replicated per the
     sharding hint).
  Phase 2 (device): per-core  Yt_i = W @ Xq_i^T  (fp16) as 288 N=512
     matmuls walking the lower triangle of W at 128-granularity; the
     stationary [128,128] W^T block is reused across 8 row-group matmuls.
     Yt written back as fp16 (halves write traffic); host adds
     b = beta - W mean, upcasts, transposes (O(N F) epilogue only).
"""
import sys

sys.path.insert(0, "/opt/trn_rl_repo")

import numpy as np
import ml_dtypes

import concourse.bass as bass
import concourse.mybir as mybir
import concourse.tile as tile
from concourse import bacc
from concourse.bass_utils import run_bass_kernel_spmd

EPS = 1e-5
N_CORES = 8
N_TOTAL = 32768
F = 1024
NC_ROWS = N_TOTAL // N_CORES  # 4096 rows per core
NT = NC_ROWS // 128           # 32 row-tiles per core
NG2 = NC_ROWS // 256          # 16 super-tiles (fp8 DoubleRow, K=256)
P = 128
FH = F // 2                   # 512
FQ = F // 4                   # 256
KB = F // P                   # 8 column blocks of 128

F32 = mybir.dt.float32
F16 = mybir.dt.float16
F8 = mybir.dt.float8e4
DR = mybir.MatmulPerfMode.DoubleRow

GRAM_MODE = "fp8dr"  # "fp16" | "fp8dr"

# fp16 gram tiles (mf, nq): rows mf*128..+128, cols nq*256..+256, ordered
# by mf; pass A = mf 0..6 (16 tiles = 8 PSUM banks x 2 halves), B = mf 7.
GRAM_TILES = sorted([(mf, nq) for nq in range(4) for mf in range(2 * nq, KB)])
GRAM_A = [t for t in GRAM_TILES if t[0] < 7]
GRAM_B = [t for t in GRAM_TILES if t[0] == 7]

# fp8dr gram tiles (mf, nh): rows mf*128..+128, cols nh*512..+512
DR_A = [(mf, 0) for mf in range(KB)]       # left half, 8 banks
DR_B1 = [(4, 1), (5, 1)]                   # lower-right quarter in three
DR_B2 = [(6, 1)]                           # passes so copies stream and
DR_B3 = [(7, 1)]                           # the final tail is one tile


def build_phase1_fp16() -> bass.Bass:
    nc = bacc.Bacc(None, target_bir_lowering=False, debug=False)

    x_in = nc.dram_tensor("x", [NC_ROWS, F], F16, kind="ExternalInput")
    gram_out = nc.dram_tensor("gram", [F, F], F32, kind="ExternalOutput")
    colsum_out = nc.dram_tensor("colsum", [P, F], F32, kind="ExternalOutput")

    with tile.TileContext(nc) as tc:
        with (
            tc.tile_pool(name="xres", bufs=1) as xres,
            tc.tile_pool(name="work", bufs=1) as work,
            tc.tile_pool(name="gout", bufs=8) as gout,
            tc.tile_pool(name="psum", bufs=8, space="PSUM") as psum,
        ):
            xt = []
            for nt in range(NT):
                t = xres.tile([P, F], F16, tag=f"x{nt}")
                if nt == 0:
                    nc.sync.dma_start(out=t[:, 0:FH], in_=x_in[0:P, 0:FH])
                    nc.sync.dma_start(out=t[:, FH:F], in_=x_in[0:P, FH:F])
                else:
                    eng = nc.sync if nt % 2 == 0 else nc.scalar
                    eng.dma_start(out=t, in_=x_in[nt * P : (nt + 1) * P, :])
                xt.append(t)

            acc = []
            for j in range(4):
                a = work.tile([P, F], F32, tag=f"acc{j}")
                nc.vector.memset(a, 0.0)
                acc.append(a)
            for nt in range(NT):
                nc.vector.tensor_add(acc[nt % 4], acc[nt % 4], xt[nt])
            nc.vector.tensor_add(acc[0], acc[0], acc[1])
            nc.vector.tensor_add(acc[2], acc[2], acc[3])
            nc.vector.tensor_add(acc[0], acc[0], acc[2])
            nc.sync.dma_start(out=colsum_out[:, :], in_=acc[0])

            for pi, tiles in enumerate([GRAM_A, GRAM_B]):
                npair = (len(tiles) + 1) // 2
                ps = [
                    psum.tile([P, 2, FQ], F32, tag="g", name=f"g_{pi}_{i}")
                    for i in range(npair)
                ]
                for nt in range(NT):
                    for i, (mf, nq) in enumerate(tiles):
                        nc.tensor.matmul(
                            ps[i % npair][:, i // npair, :],
                            xt[nt][:, mf * P : (mf + 1) * P],
                            xt[nt][:, nq * FQ : (nq + 1) * FQ],
                            start=(nt == 0 and i < npair),
                            stop=(nt == NT - 1),
                        )
                for j in range(npair):
                    for h in range(2):
                        i = j + h * npair
                        if i >= len(tiles):
                            continue
                        mf, nq = tiles[i]
                        g_sb = gout.tile(
                            [P, FQ], F32, tag="gsb", name=f"gsb_{mf}_{nq}"
                        )
                        if h == 0:
                            nc.scalar.copy(g_sb, ps[j][:, h, :])
                        else:
                            nc.vector.tensor_copy(g_sb, ps[j][:, h, :])
                        eng = nc.sync if (j + h) % 2 == 0 else nc.scalar
                        eng.dma_start(
                            out=gram_out[
                                mf * P : (mf + 1) * P, nq * FQ : (nq + 1) * FQ
                            ],
                            in_=g_sb,
                        )

    nc.compile()
    return nc


def build_phase1_fp8dr() -> bass.Bass:
    """e4m3 DoubleRow gram: K=256 per matmul, [128,512] tiles; colsum via
    ones-stationary matmuls (keeps VectorE off the critical path)."""
    nc = bacc.Bacc(None, target_bir_lowering=False, debug=False)

    x_in = nc.dram_tensor("x", [NC_ROWS, F], F8, kind="ExternalInput")
    gram_out = nc.dram_tensor("gram", [F, F], F32, kind="ExternalOutput")
    colsum_out = nc.dram_tensor("colsum", [P, F], F32, kind="ExternalOutput")

    # super-tile g holds rows g*256..(g+1)*256 as [pi, po, f], row=g*256+po*128+pi
    x_r = x_in.rearrange("(g po p) f -> p g po f", p=P, po=2)

    with tile.TileContext(nc) as tc:
        with (
            tc.tile_pool(name="xres", bufs=1) as xres,
            tc.tile_pool(name="work", bufs=1) as work,
            tc.tile_pool(name="gout", bufs=8) as gout,
            tc.tile_pool(name="psum", bufs=8, space="PSUM") as psum,
        ):
            # colsum on VectorE with two wide [P,2,F] accumulators (few,
            # big DVE ops -- per-instruction overhead dominates small adds);
            # hidden under the LDW-gated matmul stream
            acc = []
            for j in range(2):
                a = work.tile([P, 2, F], F32, tag=f"acc{j}")
                nc.vector.memset(a, 0.0)
                acc.append(a)
            csum = work.tile([P, F], F32)

            xs = []
            for g in range(NG2):
                t = xres.tile([P, 2, F], F8, tag=f"x{g}")
                if g == 0:
                    # first matmul touches only cols 0:512 of both halves
                    nc.sync.dma_start(out=t[:, :, 0:FH], in_=x_r[:, 0, :, 0:FH])
                    nc.sync.dma_start(out=t[:, :, FH:F], in_=x_r[:, 0, :, FH:F])
                else:
                    nc.sync.dma_start(out=t, in_=x_r[:, g])
                xs.append(t)
                nc.vector.tensor_add(acc[g % 2], acc[g % 2], t)
            nc.vector.tensor_add(acc[0], acc[0], acc[1])
            nc.vector.tensor_add(csum, acc[0][:, 0, :], acc[0][:, 1, :])
            nc.sync.dma_start(out=colsum_out[:, :], in_=csum)

            for pi, tiles in enumerate([DR_A, DR_B1, DR_B2, DR_B3]):
                ps = [
                    psum.tile([P, FH], F32, tag="g", name=f"g_{pi}_{i}")
                    for i in range(len(tiles))
                ]
                for g in range(NG2):
                    for i, (mf, nh) in enumerate(tiles):
                        nc.tensor.matmul(
                            ps[i],
                            xs[g][:, :, mf * P : (mf + 1) * P],
                            xs[g][:, :, nh * FH : (nh + 1) * FH],
                            start=(g == 0),
                            stop=(g == NG2 - 1),
                            perf_mode=DR,
                        )
                for i, (mf, nh) in enumerate(tiles):
                    g_sb = gout.tile([P, FH], F32, tag="gsb", name=f"gsb_{mf}_{nh}")
                    nc.scalar.copy(g_sb, ps[i])
                    eng = nc.gpsimd if i % 2 == 0 else nc.scalar
                    eng.dma_start(
                        out=gram_out[
                            mf * P : (mf + 1) * P, nh * FH : (nh + 1) * FH
                        ],
                        in_=g_sb,
                    )

    nc.compile()
    return nc


# phase-2 W^T stationary blocks: (kb, mf) with kb <= mf (W lower tri)
WT_BLOCKS = [(kb, mf) for mf in range(KB) for kb in range(mf + 1)]
WT_IDX = {bm: i for i, bm in enumerate(WT_BLOCKS)}
NRG = NC_ROWS // FH  # 8 row groups of 512


def build_phase2() -> bass.Bass:
    """Per-core: yt [F, NC_ROWS] (fp16) = W @ Xq^T  (fp16 in, no bias)."""
    nc = bacc.Bacc(None, target_bir_lowering=False, debug=False)

    xt_in = nc.dram_tensor("xt", [F, NC_ROWS], F16, kind="ExternalInput")
    wtp_in = nc.dram_tensor(
        "wtp", [P, len(WT_BLOCKS), P], F16, kind="ExternalInput"
    )
    yt_out = nc.dram_tensor("yt", [F, NC_ROWS], F16, kind="ExternalOutput")

    xt_r = xt_in.rearrange("(kb p) n -> p kb n", p=P)  # [128, 8, NC_ROWS]

    with tile.TileContext(nc) as tc:
        with (
            tc.tile_pool(name="singles", bufs=1) as singles,
            tc.tile_pool(name="yout", bufs=34) as yout,
            tc.tile_pool(name="psum", bufs=8, space="PSUM") as psum,
        ):
            xtall = singles.tile([P, KB, NC_ROWS], F16)
            wtp = singles.tile([P, len(WT_BLOCKS), P], F16)
            ww = singles.tile([P, P], F16)

            # HAM warmup: matmuls on a memset tile start right after the
            # preamble (no DMA dependency), so the PE reaches full clock
            # before the real stream and the early DMA waits are absorbed
            nc.vector.memset(ww, 0.125)
            wu = psum.tile([P, FH], F32, tag="ps", name="wu")
            NWU = 40
            for i in range(NWU):
                nc.tensor.matmul(
                    wu[:, 0:P],
                    ww,
                    ww,
                    start=(i == 0),
                    stop=(i == NWU - 1),
                )
            wu_sb = yout.tile([P, FH], F16, tag="y", name="wu_sb")
            nc.vector.tensor_copy(wu_sb[:, 0:P], wu[:, 0:P])

            # all reads share the sync ring, which drains FIFO -- exact
            # need-order priority: wtp blocks for phase mf land just
            # before xt chunk kb=mf (both gate phase mf); early chunks
            # split so matmuls unblock sooner
            def wtp_load(mf):
                lo, hi = mf * (mf + 1) // 2, (mf + 1) * (mf + 2) // 2
                nc.sync.dma_start(out=wtp[:, lo:hi, :], in_=wtp_in[:, lo:hi, :])

            wtp_load(0)
            for c0, c1 in [(0, 512), (512, 1024), (1024, 2048), (2048, 4096)]:
                nc.sync.dma_start(out=xtall[:, 0, c0:c1], in_=xt_r[:, 0, c0:c1])
            for kb in (1, 2):
                wtp_load(kb)
                for h in range(2):
                    nc.sync.dma_start(
                        out=xtall[:, kb, h * 2048 : (h + 1) * 2048],
                        in_=xt_r[:, kb, h * 2048 : (h + 1) * 2048],
                    )
            for kb in range(3, KB):
                wtp_load(kb)
                nc.sync.dma_start(out=xtall[:, kb, :], in_=xt_r[:, kb, :])

            # triangular apply: output f-block mf accumulates over kb<=mf;
            # stationary W^T block reused across the 8 row-group matmuls.
            # Early phases' writes are deferred past mf=3's matmuls so the
            # input stream gets clean HBM bandwidth; final mf runs in two
            # rg-halves so its copies/writes stream instead of piling up.
            deferred = []

            def emit_write(y_sb, mf, rg):
                eng = nc.scalar if (mf + rg) % 2 == 0 else nc.gpsimd
                eng.dma_start(
                    out=yt_out[mf * P : (mf + 1) * P, rg * FH : (rg + 1) * FH],
                    in_=y_sb,
                )

            for mf in range(KB):
                ps = [
                    psum.tile([P, FH], F32, tag="ps", name=f"ps_{mf}_{rg}")
                    for rg in range(NRG)
                ]
                # the last mf runs as two rg-halves so the first half's
                # copies/writes stream during the second half's matmuls
                rg_groups = (
                    [range(0, 4), range(4, 7), range(7, NRG)]
                    if mf == KB - 1
                    else [range(NRG)]
                )
                for rgs in rg_groups:
                    for kb in range(mf + 1):
                        w_st = wtp[:, WT_IDX[(kb, mf)], :]
                        for rg in rgs:
                            nc.tensor.matmul(
                                ps[rg],
                                w_st,
                                xtall[:, kb, rg * FH : (rg + 1) * FH],
                                start=(kb == 0),
                                stop=(kb == mf),
                            )
                if mf == 3:
                    # guard: scalar waits for the last read chunk before
                    # issuing the deferred writes, keeping HBM bandwidth
                    # clean for the input stream
                    guard = yout.tile([P, FH], F16, tag="y", name="guard")
                    nc.gpsimd.tensor_copy(guard[:, 0:8], xtall[:, KB - 1, 4088:4096])
                    for y_sb, dmf, drg in deferred:
                        nc.gpsimd.dma_start(
                            out=yt_out[
                                dmf * P : (dmf + 1) * P,
                                drg * FH : (drg + 1) * FH,
                            ],
                            in_=y_sb,
                        )
                    deferred = []
                for rg in range(NRG):
                    y_sb = yout.tile([P, FH], F16, tag="y", name=f"y_{mf}_{rg}")
                    if rg % 2 == 0:
                        nc.vector.tensor_copy(y_sb, ps[rg])
                    else:
                        nc.scalar.copy(y_sb, ps[rg])
                    if mf <= 2:
                        deferred.append((y_sb, mf, rg))
                    else:
                        emit_write(y_sb, mf, rg)

    nc.compile()
    return nc


_programs: dict = {}


def _get_programs():
    if "p1" not in _programs:
        _programs["p1"] = (
            build_phase1_fp8dr() if GRAM_MODE == "fp8dr" else build_phase1_fp16()
        )
        _programs["p2"] = build_phase2()
    return _programs["p1"], _programs["p2"]


def kernel(X, running_mean, running_cov, beta, trace=False):
    X = np.asarray(X, dtype=np.float32)
    beta = np.asarray(beta, dtype=np.float32)
    assert X.shape == (N_TOTAL, F)

    p1, p2 = _get_programs()
    core_ids = list(range(N_CORES))

    p1_dt = ml_dtypes.float8_e4m3 if GRAM_MODE == "fp8dr" else np.float16
    shards1 = X.astype(p1_dt).reshape(N_CORES, NC_ROWS, F)

    tkw = {"trace_cores": core_ids} if trace else {}

    def _run(prog, in_maps):
        try:
            return run_bass_kernel_spmd(prog, in_maps, core_ids, trace=trace, **tkw)
        except Exception:
            import time as _time

            _time.sleep(2.0)
            return run_bass_kernel_spmd(prog, in_maps, core_ids, trace=trace, **tkw)

    in1 = [{"x": np.ascontiguousarray(shards1[i])} for i in range(N_CORES)]
    r1 = _run(p1, in1)
    kernel.exec_ns_phase1 = r1.exec_time_ns

    gram = np.zeros((F, F), dtype=np.float64)
    colsum = np.zeros((F,), dtype=np.float64)
    for res in r1.results:
        gram += res["gram"].astype(np.float64)
        colsum += res["colsum"].astype(np.float64).sum(axis=0)
    # mirror the computed lower triangle onto the upper
    gram = np.tril(gram) + np.tril(gram, -1).T

    mean = colsum / N_TOTAL
    cov = gram / N_TOTAL - np.outer(mean, mean)
    a = cov + EPS * np.eye(F, dtype=np.float64)
    L = np.linalg.cholesky(a)
    w = np.linalg.solve(L, np.eye(F, dtype=np.float64))  # W = L^-1
    wt = w.T  # upper triangular [k, f]
    wtp = np.zeros((P, len(WT_BLOCKS), P), dtype=np.float16)
    for (kb, mf), i in WT_IDX.items():
        wtp[:, i, :] = wt[kb * P : (kb + 1) * P, mf * P : (mf + 1) * P].astype(
            np.float16
        )
    b = (beta.astype(np.float64) - w @ mean).astype(np.float32)

    xts = np.ascontiguousarray(
        X.astype(np.float16).reshape(N_CORES, NC_ROWS, F).transpose(0, 2, 1)
    )
    in2 = [{"xt": xts[i], "wtp": wtp} for i in range(N_CORES)]
    r2 = _run(p2, in2)
    kernel.exec_ns_phase2 = r2.exec_time_ns

    # host epilogue: bias + upcast + transpose back (O(N F))
    y = np.empty((N_TOTAL, F), dtype=np.float32)
    for i, res in enumerate(r2.results):
        y[i * NC_ROWS : (i + 1) * NC_ROWS, :] = (
            res["yt"].astype(np.float32) + b[:, None]
        ).T
    return y


kernel.exec_ns_phase1 = None
kernel.exec_ns_phase2 = None


# revision 37
# speedup vs baseline: 1.0244x; 1.0244x over previous
"""Batch whitening (Cholesky) kernel for Trainium2, 8 NeuronCores.

Computes, for X [32768, 1024] (matching the reference nn_BWCholeskyBlock):
    mean = X.mean(0); xc = X - mean; cov = xc.T @ xc / N
    L = chol(cov + eps I);  Y = (L^-1 xc^T).T + beta

Strategy (data-parallel over batch, 8 cores; harness gate rel_err < 2e-2):
  Phase 1 (device): per-core partial gram  G_i = Xq_i^T Xq_i  and column
     sums.  Two dtype modes:
       fp16:  [128,256] gram tiles                   (~72us MM stream)
       fp8dr: e4m3 DoubleRow [128,512] gram tiles, K=256 per matmul --
              runs at the fp8 PE peak               (~44us MM stream)
     Column sums ride on VectorE (wide [P,2,F] fp32 accumulators),
     fully hidden under the matmul stream.
  Host: reduce partials, mirror the triangle -> mean, cov; Cholesky +
     triangular inverse of the small [F,F] factor (replicated per the
     sharding hint).
  Phase 2 (device): per-core  Yt_i = W @ Xq_i^T  (fp16) as 288 N=512
     matmuls walking the lower triangle of W at 128-granularity; the
     stationary [128,128] W^T block is reused across 8 row-group matmuls.
     Yt written back as fp16 (halves write traffic); host adds
     b = beta - W mean, upcasts, transposes (O(N F) epilogue only).
"""
import sys

sys.path.insert(0, "/opt/trn_rl_repo")

import numpy as np
import ml_dtypes

import concourse.bass as bass
import concourse.mybir as mybir
import concourse.tile as tile
from concourse import bacc
from concourse.bass_utils import run_bass_kernel_spmd

EPS = 1e-5
N_CORES = 8
N_TOTAL = 32768
F = 1024
NC_ROWS = N_TOTAL // N_CORES  # 4096 rows per core
NT = NC_ROWS // 128           # 32 row-tiles per core
NG2 = NC_ROWS // 256          # 16 super-tiles (fp8 DoubleRow, K=256)
P = 128
FH = F // 2                   # 512
FQ = F // 4                   # 256
KB = F // P                   # 8 column blocks of 128

F32 = mybir.dt.float32
F16 = mybir.dt.float16
F8 = mybir.dt.float8e4
DR = mybir.MatmulPerfMode.DoubleRow

GRAM_MODE = "fp8dr"  # "fp16" | "fp8dr"

# fp16 gram tiles (mf, nq): rows mf*128..+128, cols nq*256..+256, ordered
# by mf; pass A = mf 0..6 (16 tiles = 8 PSUM banks x 2 halves), B = mf 7.
GRAM_TILES = sorted([(mf, nq) for nq in range(4) for mf in range(2 * nq, KB)])
GRAM_A = [t for t in GRAM_TILES if t[0] < 7]
GRAM_B = [t for t in GRAM_TILES if t[0] == 7]

# fp8dr gram tiles (mf, nh): rows mf*128..+128, cols nh*512..+512
DR_A = [(mf, 0) for mf in range(KB)]       # left half, 8 banks
DR_B1 = [(4, 1), (5, 1)]                   # lower-right quarter in three
DR_B2 = [(6, 1)]                           # passes so copies stream and
DR_B3 = [(7, 1)]                           # the final tail is one tile


def build_phase1_fp16() -> bass.Bass:
    nc = bacc.Bacc(None, target_bir_lowering=False, debug=False)

    x_in = nc.dram_tensor("x", [NC_ROWS, F], F16, kind="ExternalInput")
    gram_out = nc.dram_tensor("gram", [F, F], F32, kind="ExternalOutput")
    colsum_out = nc.dram_tensor("colsum", [P, F], F32, kind="ExternalOutput")

    with tile.TileContext(nc) as tc:
        with (
            tc.tile_pool(name="xres", bufs=1) as xres,
            tc.tile_pool(name="work", bufs=1) as work,
            tc.tile_pool(name="gout", bufs=8) as gout,
            tc.tile_pool(name="psum", bufs=8, space="PSUM") as psum,
        ):
            xt = []
            for nt in range(NT):
                t = xres.tile([P, F], F16, tag=f"x{nt}")
                if nt == 0:
                    nc.sync.dma_start(out=t[:, 0:FH], in_=x_in[0:P, 0:FH])
                    nc.sync.dma_start(out=t[:, FH:F], in_=x_in[0:P, FH:F])
                else:
                    eng = nc.sync if nt % 2 == 0 else nc.scalar
                    eng.dma_start(out=t, in_=x_in[nt * P : (nt + 1) * P, :])
                xt.append(t)

            acc = []
            for j in range(4):
                a = work.tile([P, F], F32, tag=f"acc{j}")
                nc.vector.memset(a, 0.0)
                acc.append(a)
            for nt in range(NT):
                nc.vector.tensor_add(acc[nt % 4], acc[nt % 4], xt[nt])
            nc.vector.tensor_add(acc[0], acc[0], acc[1])
            nc.vector.tensor_add(acc[2], acc[2], acc[3])
            nc.vector.tensor_add(acc[0], acc[0], acc[2])
            nc.sync.dma_start(out=colsum_out[:, :], in_=acc[0])

            for pi, tiles in enumerate([GRAM_A, GRAM_B]):
                npair = (len(tiles) + 1) // 2
                ps = [
                    psum.tile([P, 2, FQ], F32, tag="g", name=f"g_{pi}_{i}")
                    for i in range(npair)
                ]
                for nt in range(NT):
                    for i, (mf, nq) in enumerate(tiles):
                        nc.tensor.matmul(
                            ps[i % npair][:, i // npair, :],
                            xt[nt][:, mf * P : (mf + 1) * P],
                            xt[nt][:, nq * FQ : (nq + 1) * FQ],
                            start=(nt == 0 and i < npair),
                            stop=(nt == NT - 1),
                        )
                for j in range(npair):
                    for h in range(2):
                        i = j + h * npair
                        if i >= len(tiles):
                            continue
                        mf, nq = tiles[i]
                        g_sb = gout.tile(
                            [P, FQ], F32, tag="gsb", name=f"gsb_{mf}_{nq}"
                        )
                        if h == 0:
                            nc.scalar.copy(g_sb, ps[j][:, h, :])
                        else:
                            nc.vector.tensor_copy(g_sb, ps[j][:, h, :])
                        eng = nc.sync if (j + h) % 2 == 0 else nc.scalar
                        eng.dma_start(
                            out=gram_out[
                                mf * P : (mf + 1) * P, nq * FQ : (nq + 1) * FQ
                            ],
                            in_=g_sb,
                        )

    nc.compile()
    return nc


def build_phase1_fp8dr() -> bass.Bass:
    """e4m3 DoubleRow gram: K=256 per matmul, [128,512] tiles; colsum via
    ones-stationary matmuls (keeps VectorE off the critical path)."""
    nc = bacc.Bacc(None, target_bir_lowering=False, debug=False)

    x_in = nc.dram_tensor("x", [NC_ROWS, F], F8, kind="ExternalInput")
    gram_out = nc.dram_tensor("gram", [F, F], F32, kind="ExternalOutput")
    colsum_out = nc.dram_tensor("colsum", [P, F], F32, kind="ExternalOutput")

    # super-tile g holds rows g*256..(g+1)*256 as [pi, po, f], row=g*256+po*128+pi
    x_r = x_in.rearrange("(g po p) f -> p g po f", p=P, po=2)

    with tile.TileContext(nc) as tc:
        with (
            tc.tile_pool(name="xres", bufs=1) as xres,
            tc.tile_pool(name="work", bufs=1) as work,
            tc.tile_pool(name="gout", bufs=8) as gout,
            tc.tile_pool(name="psum", bufs=8, space="PSUM") as psum,
        ):
            # colsum on VectorE with two wide [P,2,F] accumulators (few,
            # big DVE ops -- per-instruction overhead dominates small adds);
            # hidden under the LDW-gated matmul stream
            acc = []
            for j in range(2):
                a = work.tile([P, 2, F], F32, tag=f"acc{j}")
                nc.vector.memset(a, 0.0)
                acc.append(a)
            csum = work.tile([P, F], F32)

            xs = []
            for g in range(NG2):
                t = xres.tile([P, 2, F], F8, tag=f"x{g}")
                if g == 0:
                    # first matmul touches only cols 0:512 of both halves
                    nc.sync.dma_start(out=t[:, :, 0:FH], in_=x_r[:, 0, :, 0:FH])
                    nc.sync.dma_start(out=t[:, :, FH:F], in_=x_r[:, 0, :, FH:F])
                else:
                    nc.sync.dma_start(out=t, in_=x_r[:, g])
                xs.append(t)
                nc.vector.tensor_add(acc[g % 2], acc[g % 2], t)
            nc.vector.tensor_add(acc[0], acc[0], acc[1])
            nc.vector.tensor_add(csum, acc[0][:, 0, :], acc[0][:, 1, :])
            nc.sync.dma_start(out=colsum_out[:, :], in_=csum)

            for pi, tiles in enumerate([DR_A, DR_B1, DR_B2, DR_B3]):
                ps = [
                    psum.tile([P, FH], F32, tag="g", name=f"g_{pi}_{i}")
                    for i in range(len(tiles))
                ]
                for g in range(NG2):
                    for i, (mf, nh) in enumerate(tiles):
                        nc.tensor.matmul(
                            ps[i],
                            xs[g][:, :, mf * P : (mf + 1) * P],
                            xs[g][:, :, nh * FH : (nh + 1) * FH],
                            start=(g == 0),
                            stop=(g == NG2 - 1),
                            perf_mode=DR,
                        )
                for i, (mf, nh) in enumerate(tiles):
                    g_sb = gout.tile([P, FH], F32, tag="gsb", name=f"gsb_{mf}_{nh}")
                    nc.scalar.copy(g_sb, ps[i])
                    eng = nc.gpsimd if i % 2 == 0 else nc.scalar
                    eng.dma_start(
                        out=gram_out[
                            mf * P : (mf + 1) * P, nh * FH : (nh + 1) * FH
                        ],
                        in_=g_sb,
                    )

    nc.compile()
    return nc


# phase-2 W^T stationary blocks: (kb, mf) with kb <= mf (W lower tri)
WT_BLOCKS = [(kb, mf) for mf in range(KB) for kb in range(mf + 1)]
WT_IDX = {bm: i for i, bm in enumerate(WT_BLOCKS)}
NRG = NC_ROWS // FH  # 8 row groups of 512


def build_phase2() -> bass.Bass:
    """Per-core: yt [F, NC_ROWS] (fp16) = W @ Xq^T  (fp16 in, no bias)."""
    nc = bacc.Bacc(None, target_bir_lowering=False, debug=False)

    xt_in = nc.dram_tensor("xt", [F, NC_ROWS], F16, kind="ExternalInput")
    wtp_in = nc.dram_tensor(
        "wtp", [P, len(WT_BLOCKS), P], F16, kind="ExternalInput"
    )
    yt_out = nc.dram_tensor("yt", [F, NC_ROWS], F16, kind="ExternalOutput")

    xt_r = xt_in.rearrange("(kb p) n -> p kb n", p=P)  # [128, 8, NC_ROWS]

    with tile.TileContext(nc) as tc:
        with (
            tc.tile_pool(name="singles", bufs=1) as singles,
            tc.tile_pool(name="yout", bufs=34) as yout,
            tc.tile_pool(name="psum", bufs=8, space="PSUM") as psum,
        ):
            xtall = singles.tile([P, KB, NC_ROWS], F16)
            wtp = singles.tile([P, len(WT_BLOCKS), P], F16)
            ww = singles.tile([P, P], F16)

            # HAM warmup: matmuls on a memset tile start right after the
            # preamble (no DMA dependency), so the PE reaches full clock
            # before the real stream and the early DMA waits are absorbed
            nc.vector.memset(ww, 0.125)
            wu = psum.tile([P, FH], F32, tag="ps", name="wu")
            NWU = 40
            for i in range(NWU):
                nc.tensor.matmul(
                    wu[:, 0:P],
                    ww,
                    ww,
                    start=(i == 0),
                    stop=(i == NWU - 1),
                )
            wu_sb = yout.tile([P, FH], F16, tag="y", name="wu_sb")
            nc.vector.tensor_copy(wu_sb[:, 0:P], wu[:, 0:P])

            # all reads share the sync ring, which drains FIFO -- exact
            # need-order priority: wtp blocks for phase mf land just
            # before xt chunk kb=mf (both gate phase mf); early chunks
            # split so matmuls unblock sooner
            def wtp_load(mf):
                lo, hi = mf * (mf + 1) // 2, (mf + 1) * (mf + 2) // 2
                nc.sync.dma_start(out=wtp[:, lo:hi, :], in_=wtp_in[:, lo:hi, :])

            wtp_load(0)
            for c0, c1 in [(0, 512), (512, 1024), (1024, 2048), (2048, 4096)]:
                nc.sync.dma_start(out=xtall[:, 0, c0:c1], in_=xt_r[:, 0, c0:c1])
            for kb in (1, 2):
                wtp_load(kb)
                for h in range(2):
                    nc.sync.dma_start(
                        out=xtall[:, kb, h * 2048 : (h + 1) * 2048],
                        in_=xt_r[:, kb, h * 2048 : (h + 1) * 2048],
                    )
            for kb in range(3, KB):
                wtp_load(kb)
                nc.sync.dma_start(out=xtall[:, kb, :], in_=xt_r[:, kb, :])

            # triangular apply: output f-block mf accumulates over kb<=mf;
            # stationary W^T block reused across the 8 row-group matmuls.
            # Early phases' writes are deferred past mf=3's matmuls so the
            # input stream gets clean HBM bandwidth; final mf runs in two
            # rg-halves so its copies/writes stream instead of piling up.
            deferred = []

            def emit_write(y_sb, mf, rg):
                eng = nc.scalar if (mf + rg) % 2 == 0 else nc.gpsimd
                eng.dma_start(
                    out=yt_out[mf * P : (mf + 1) * P, rg * FH : (rg + 1) * FH],
                    in_=y_sb,
                )

            for mf in range(KB):
                ps = [
                    psum.tile([P, FH], F32, tag="ps", name=f"ps_{mf}_{rg}")
                    for rg in range(NRG)
                ]
                # the last mf runs as two rg-halves so the first half's
                # copies/writes stream during the second half's matmuls
                rg_groups = (
                    [range(0, 4), range(4, 7), range(7, NRG)]
                    if mf == KB - 1
                    else [range(NRG)]
                )
                for rgs in rg_groups:
                    for kb in range(mf + 1):
                        w_st = wtp[:, WT_IDX[(kb, mf)], :]
                        for rg in rgs:
                            nc.tensor.matmul(
                                ps[rg],
                                w_st,
                                xtall[:, kb, rg * FH : (rg + 1) * FH],
                                start=(kb == 0),
                                stop=(kb == mf),
                            )
                if mf == 3:
                    # guard: scalar waits for the last read chunk before
                    # issuing the deferred writes, keeping HBM bandwidth
                    # clean for the input stream
                    guard = yout.tile([P, FH], F16, tag="y", name="guard")
                    nc.gpsimd.tensor_copy(guard[:, 0:8], xtall[:, KB - 1, 4088:4096])
                    for y_sb, dmf, drg in deferred:
                        nc.gpsimd.dma_start(
                            out=yt_out[
                                dmf * P : (dmf + 1) * P,
                                drg * FH : (drg + 1) * FH,
                            ],
                            in_=y_sb,
                        )
                    deferred = []
                for rg in range(NRG):
                    y_sb = yout.tile([P, FH], F16, tag="y", name=f"y_{mf}_{rg}")
                    if rg % 2 == 0:
                        nc.vector.tensor_copy(y_sb, ps[rg])
                    else:
                        nc.scalar.copy(y_sb, ps[rg])
                    if mf <= 2:
                        deferred.append((y_sb, mf, rg))
                    else:
                        emit_write(y_sb, mf, rg)

    nc.compile()
    return nc


_programs: dict = {}


def _get_programs():
    if "p1" not in _programs:
        _programs["p1"] = (
            build_phase1_fp8dr() if GRAM_MODE == "fp8dr" else build_phase1_fp16()
        )
        _programs["p2"] = build_phase2()
    return _programs["p1"], _programs["p2"]


def kernel(X, running_mean, running_cov, beta, trace=False):
    X = np.asarray(X, dtype=np.float32)
    beta = np.asarray(beta, dtype=np.float32)
    assert X.shape == (N_TOTAL, F)

    p1, p2 = _get_programs()
    core_ids = list(range(N_CORES))

    p1_dt = ml_dtypes.float8_e4m3 if GRAM_MODE == "fp8dr" else np.float16
    shards1 = X.astype(p1_dt).reshape(N_CORES, NC_ROWS, F)

    tkw = {"trace_cores": core_ids} if trace else {}

    def _run(prog, in_maps):
        # transient NRT/device hiccups (incl. NRT_EXEC_UNIT_UNRECOVERABLE
        # wedges) have been observed; back off and retry
        import time as _time

        for attempt, pause in enumerate((2.0, 10.0, 30.0)):
            try:
                return run_bass_kernel_spmd(
                    prog, in_maps, core_ids, trace=trace, **tkw
                )
            except Exception:
                _time.sleep(pause)
        return run_bass_kernel_spmd(prog, in_maps, core_ids, trace=trace, **tkw)

    in1 = [{"x": np.ascontiguousarray(shards1[i])} for i in range(N_CORES)]
    r1 = _run(p1, in1)
    kernel.exec_ns_phase1 = r1.exec_time_ns

    gram = np.zeros((F, F), dtype=np.float64)
    colsum = np.zeros((F,), dtype=np.float64)
    for res in r1.results:
        gram += res["gram"].astype(np.float64)
        colsum += res["colsum"].astype(np.float64).sum(axis=0)
    # mirror the computed lower triangle onto the upper
    gram = np.tril(gram) + np.tril(gram, -1).T

    mean = colsum / N_TOTAL
    cov = gram / N_TOTAL - np.outer(mean, mean)
    a = cov + EPS * np.eye(F, dtype=np.float64)
    L = np.linalg.cholesky(a)
    w = np.linalg.solve(L, np.eye(F, dtype=np.float64))  # W = L^-1
    wt = w.T  # upper triangular [k, f]
    wtp = np.zeros((P, len(WT_BLOCKS), P), dtype=np.float16)
    for (kb, mf), i in WT_IDX.items():
        wtp[:, i, :] = wt[kb * P : (kb + 1) * P, mf * P : (mf + 1) * P].astype(
            np.float16
        )
    b = (beta.astype(np.float64) - w @ mean).astype(np.float32)

    xts = np.ascontiguousarray(
        X.astype(np.float16).reshape(N_CORES, NC_ROWS, F).transpose(0, 2, 1)
    )
    in2 = [{"xt": xts[i], "wtp": wtp} for i in range(N_CORES)]
    r2 = _run(p2, in2)
    kernel.exec_ns_phase2 = r2.exec_time_ns

    # host epilogue: bias + upcast + transpose back (O(N F))
    y = np.empty((N_TOTAL, F), dtype=np.float32)
    for i, res in enumerate(r2.results):
        y[i * NC_ROWS : (i + 1) * NC_ROWS, :] = (
            res["yt"].astype(np.float32) + b[:, None]
        ).T
    return y


kernel.exec_ns_phase1 = None
kernel.exec_ns_phase2 = None


# revision 39
# speedup vs baseline: 1.0280x; 1.0035x over previous
"""Batch whitening (Cholesky) kernel for Trainium2, 8 NeuronCores.

Computes, for X [32768, 1024] (matching the reference nn_BWCholeskyBlock):
    mean = X.mean(0); xc = X - mean; cov = xc.T @ xc / N
    L = chol(cov + eps I);  Y = (L^-1 xc^T).T + beta

Strategy (data-parallel over batch, 8 cores; harness gate rel_err < 2e-2):
  Phase 1 (device): per-core partial gram  G_i = Xq_i^T Xq_i  and column
     sums.  Two dtype modes:
       fp16:  [128,256] gram tiles                   (~72us MM stream)
       fp8dr: e4m3 DoubleRow [128,512] gram tiles, K=256 per matmul --
              runs at the fp8 PE peak               (~44us MM stream)
     Column sums ride on VectorE (wide [P,2,F] fp32 accumulators),
     fully hidden under the matmul stream.
  Host: reduce partials, mirror the triangle -> mean, cov; Cholesky +
     triangular inverse of the small [F,F] factor (replicated per the
     sharding hint).
  Phase 2 (device): per-core  Yt_i = W @ Xq_i^T  (fp16) as 288 N=512
     matmuls walking the lower triangle of W at 128-granularity; the
     stationary [128,128] W^T block is reused across 8 row-group matmuls.
     Yt written back as fp16 (halves write traffic); host adds
     b = beta - W mean, upcasts, transposes (O(N F) epilogue only).
"""
import sys

sys.path.insert(0, "/opt/trn_rl_repo")

import numpy as np
import ml_dtypes

import concourse.bass as bass
import concourse.mybir as mybir
import concourse.tile as tile
from concourse import bacc
from concourse.bass_utils import run_bass_kernel_spmd

EPS = 1e-5
N_CORES = 8
N_TOTAL = 32768
F = 1024
NC_ROWS = N_TOTAL // N_CORES  # 4096 rows per core
NT = NC_ROWS // 128           # 32 row-tiles per core
NG2 = NC_ROWS // 256          # 16 super-tiles (fp8 DoubleRow, K=256)
P = 128
FH = F // 2                   # 512
FQ = F // 4                   # 256
KB = F // P                   # 8 column blocks of 128

F32 = mybir.dt.float32
F16 = mybir.dt.float16
F8 = mybir.dt.float8e4
DR = mybir.MatmulPerfMode.DoubleRow

GRAM_MODE = "fp8dr"  # "fp16" | "fp8dr"

# fp16 gram tiles (mf, nq): rows mf*128..+128, cols nq*256..+256, ordered
# by mf; pass A = mf 0..6 (16 tiles = 8 PSUM banks x 2 halves), B = mf 7.
GRAM_TILES = sorted([(mf, nq) for nq in range(4) for mf in range(2 * nq, KB)])
GRAM_A = [t for t in GRAM_TILES if t[0] < 7]
GRAM_B = [t for t in GRAM_TILES if t[0] == 7]

# fp8dr gram tiles (mf, nh): rows mf*128..+128, cols nh*512..+512
DR_A = [(mf, 0) for mf in range(KB)]       # left half, 8 banks
DR_B1 = [(4, 1), (5, 1)]                   # lower-right quarter in three
DR_B2 = [(6, 1)]                           # passes so copies stream and
DR_B3 = [(7, 1)]                           # the final tail is one tile


def build_phase1_fp16() -> bass.Bass:
    nc = bacc.Bacc(None, target_bir_lowering=False, debug=False)

    x_in = nc.dram_tensor("x", [NC_ROWS, F], F16, kind="ExternalInput")
    gram_out = nc.dram_tensor("gram", [F, F], F32, kind="ExternalOutput")
    colsum_out = nc.dram_tensor("colsum", [P, F], F32, kind="ExternalOutput")

    with tile.TileContext(nc) as tc:
        with (
            tc.tile_pool(name="xres", bufs=1) as xres,
            tc.tile_pool(name="work", bufs=1) as work,
            tc.tile_pool(name="gout", bufs=8) as gout,
            tc.tile_pool(name="psum", bufs=8, space="PSUM") as psum,
        ):
            xt = []
            for nt in range(NT):
                t = xres.tile([P, F], F16, tag=f"x{nt}")
                if nt == 0:
                    nc.sync.dma_start(out=t[:, 0:FH], in_=x_in[0:P, 0:FH])
                    nc.sync.dma_start(out=t[:, FH:F], in_=x_in[0:P, FH:F])
                else:
                    eng = nc.sync if nt % 2 == 0 else nc.scalar
                    eng.dma_start(out=t, in_=x_in[nt * P : (nt + 1) * P, :])
                xt.append(t)

            acc = []
            for j in range(4):
                a = work.tile([P, F], F32, tag=f"acc{j}")
                nc.vector.memset(a, 0.0)
                acc.append(a)
            for nt in range(NT):
                nc.vector.tensor_add(acc[nt % 4], acc[nt % 4], xt[nt])
            nc.vector.tensor_add(acc[0], acc[0], acc[1])
            nc.vector.tensor_add(acc[2], acc[2], acc[3])
            nc.vector.tensor_add(acc[0], acc[0], acc[2])
            nc.sync.dma_start(out=colsum_out[:, :], in_=acc[0])

            for pi, tiles in enumerate([GRAM_A, GRAM_B]):
                npair = (len(tiles) + 1) // 2
                ps = [
                    psum.tile([P, 2, FQ], F32, tag="g", name=f"g_{pi}_{i}")
                    for i in range(npair)
                ]
                for nt in range(NT):
                    for i, (mf, nq) in enumerate(tiles):
                        nc.tensor.matmul(
                            ps[i % npair][:, i // npair, :],
                            xt[nt][:, mf * P : (mf + 1) * P],
                            xt[nt][:, nq * FQ : (nq + 1) * FQ],
                            start=(nt == 0 and i < npair),
                            stop=(nt == NT - 1),
                        )
                for j in range(npair):
                    for h in range(2):
                        i = j + h * npair
                        if i >= len(tiles):
                            continue
                        mf, nq = tiles[i]
                        g_sb = gout.tile(
                            [P, FQ], F32, tag="gsb", name=f"gsb_{mf}_{nq}"
                        )
                        if h == 0:
                            nc.scalar.copy(g_sb, ps[j][:, h, :])
                        else:
                            nc.vector.tensor_copy(g_sb, ps[j][:, h, :])
                        eng = nc.sync if (j + h) % 2 == 0 else nc.scalar
                        eng.dma_start(
                            out=gram_out[
                                mf * P : (mf + 1) * P, nq * FQ : (nq + 1) * FQ
                            ],
                            in_=g_sb,
                        )

    nc.compile()
    return nc


def build_phase1_fp8dr() -> bass.Bass:
    """e4m3 DoubleRow gram: K=256 per matmul, [128,512] tiles; colsum via
    ones-stationary matmuls (keeps VectorE off the critical path)."""
    nc = bacc.Bacc(None, target_bir_lowering=False, debug=False)

    x_in = nc.dram_tensor("x", [NC_ROWS, F], F8, kind="ExternalInput")
    gram_out = nc.dram_tensor("gram", [F, F], F32, kind="ExternalOutput")
    colsum_out = nc.dram_tensor("colsum", [P, F], F32, kind="ExternalOutput")

    # super-tile g holds rows g*256..(g+1)*256 as [pi, po, f], row=g*256+po*128+pi
    x_r = x_in.rearrange("(g po p) f -> p g po f", p=P, po=2)

    with tile.TileContext(nc) as tc:
        with (
            tc.tile_pool(name="xres", bufs=1) as xres,
            tc.tile_pool(name="work", bufs=1) as work,
            tc.tile_pool(name="gout", bufs=8) as gout,
            tc.tile_pool(name="psum", bufs=8, space="PSUM") as psum,
        ):
            # short HAM warmup on a memset tile (no DMA dependency): ~3.4us
            # of PE busy ending as the first x chunk lands, so the DR
            # stream starts at full clock
            ww = work.tile([P, FH], F8)
            nc.vector.memset(ww, 0.125)
            wu = psum.tile([P, FH], F32, tag="g", name="wu")
            NWU = 8
            for i in range(NWU):
                nc.tensor.matmul(
                    wu,
                    ww[:, 0:P],
                    ww,
                    start=(i == 0),
                    stop=(i == NWU - 1),
                )
            wu_sb = gout.tile([P, FH], F32, tag="gsb", name="wu_sb")
            nc.scalar.copy(wu_sb[:, 0:P], wu[:, 0:P])

            # colsum on VectorE with two wide [P,2,F] accumulators (few,
            # big DVE ops -- per-instruction overhead dominates small adds);
            # hidden under the LDW-gated matmul stream
            acc = []
            for j in range(2):
                a = work.tile([P, 2, F], F32, tag=f"acc{j}")
                nc.vector.memset(a, 0.0)
                acc.append(a)
            csum = work.tile([P, F], F32)

            xs = []
            for g in range(NG2):
                t = xres.tile([P, 2, F], F8, tag=f"x{g}")
                if g == 0:
                    # first matmul touches only cols 0:512 of both halves
                    nc.sync.dma_start(out=t[:, :, 0:FH], in_=x_r[:, 0, :, 0:FH])
                    nc.sync.dma_start(out=t[:, :, FH:F], in_=x_r[:, 0, :, FH:F])
                else:
                    nc.sync.dma_start(out=t, in_=x_r[:, g])
                xs.append(t)
                nc.vector.tensor_add(acc[g % 2], acc[g % 2], t)
            nc.vector.tensor_add(acc[0], acc[0], acc[1])
            nc.vector.tensor_add(csum, acc[0][:, 0, :], acc[0][:, 1, :])
            nc.sync.dma_start(out=colsum_out[:, :], in_=csum)

            for pi, tiles in enumerate([DR_A, DR_B1, DR_B2, DR_B3]):
                ps = [
                    psum.tile([P, FH], F32, tag="g", name=f"g_{pi}_{i}")
                    for i in range(len(tiles))
                ]
                for g in range(NG2):
                    for i, (mf, nh) in enumerate(tiles):
                        nc.tensor.matmul(
                            ps[i],
                            xs[g][:, :, mf * P : (mf + 1) * P],
                            xs[g][:, :, nh * FH : (nh + 1) * FH],
                            start=(g == 0),
                            stop=(g == NG2 - 1),
                            perf_mode=DR,
                        )
                for i, (mf, nh) in enumerate(tiles):
                    g_sb = gout.tile([P, FH], F32, tag="gsb", name=f"gsb_{mf}_{nh}")
                    nc.scalar.copy(g_sb, ps[i])
                    eng = nc.gpsimd if i % 2 == 0 else nc.scalar
                    eng.dma_start(
                        out=gram_out[
                            mf * P : (mf + 1) * P, nh * FH : (nh + 1) * FH
                        ],
                        in_=g_sb,
                    )

    nc.compile()
    return nc


# phase-2 W^T stationary blocks: (kb, mf) with kb <= mf (W lower tri)
WT_BLOCKS = [(kb, mf) for mf in range(KB) for kb in range(mf + 1)]
WT_IDX = {bm: i for i, bm in enumerate(WT_BLOCKS)}
NRG = NC_ROWS // FH  # 8 row groups of 512


def build_phase2() -> bass.Bass:
    """Per-core: yt [F, NC_ROWS] (fp16) = W @ Xq^T  (fp16 in, no bias)."""
    nc = bacc.Bacc(None, target_bir_lowering=False, debug=False)

    xt_in = nc.dram_tensor("xt", [F, NC_ROWS], F16, kind="ExternalInput")
    wtp_in = nc.dram_tensor(
        "wtp", [P, len(WT_BLOCKS), P], F16, kind="ExternalInput"
    )
    yt_out = nc.dram_tensor("yt", [F, NC_ROWS], F16, kind="ExternalOutput")

    xt_r = xt_in.rearrange("(kb p) n -> p kb n", p=P)  # [128, 8, NC_ROWS]

    with tile.TileContext(nc) as tc:
        with (
            tc.tile_pool(name="singles", bufs=1) as singles,
            tc.tile_pool(name="yout", bufs=34) as yout,
            tc.tile_pool(name="psum", bufs=8, space="PSUM") as psum,
        ):
            xtall = singles.tile([P, KB, NC_ROWS], F16)
            wtp = singles.tile([P, len(WT_BLOCKS), P], F16)
            ww = singles.tile([P, P], F16)

            # HAM warmup: matmuls on a memset tile start right after the
            # preamble (no DMA dependency), so the PE reaches full clock
            # before the real stream and the early DMA waits are absorbed
            nc.vector.memset(ww, 0.125)
            wu = psum.tile([P, FH], F32, tag="ps", name="wu")
            NWU = 56
            for i in range(NWU):
                nc.tensor.matmul(
                    wu[:, 0:P],
                    ww,
                    ww,
                    start=(i == 0),
                    stop=(i == NWU - 1),
                )
            wu_sb = yout.tile([P, FH], F16, tag="y", name="wu_sb")
            nc.vector.tensor_copy(wu_sb[:, 0:P], wu[:, 0:P])

            # all reads share the sync ring, which drains FIFO -- exact
            # need-order priority: wtp blocks for phase mf land just
            # before xt chunk kb=mf (both gate phase mf); early chunks
            # split so matmuls unblock sooner
            def wtp_load(mf):
                lo, hi = mf * (mf + 1) // 2, (mf + 1) * (mf + 2) // 2
                nc.sync.dma_start(out=wtp[:, lo:hi, :], in_=wtp_in[:, lo:hi, :])

            wtp_load(0)
            for c0, c1 in [(0, 512), (512, 1024), (1024, 2048), (2048, 4096)]:
                nc.sync.dma_start(out=xtall[:, 0, c0:c1], in_=xt_r[:, 0, c0:c1])
            for kb in (1, 2):
                wtp_load(kb)
                for h in range(2):
                    nc.sync.dma_start(
                        out=xtall[:, kb, h * 2048 : (h + 1) * 2048],
                        in_=xt_r[:, kb, h * 2048 : (h + 1) * 2048],
                    )
            for kb in range(3, KB):
                wtp_load(kb)
                nc.sync.dma_start(out=xtall[:, kb, :], in_=xt_r[:, kb, :])

            # triangular apply: output f-block mf accumulates over kb<=mf;
            # stationary W^T block reused across the 8 row-group matmuls.
            # Early phases' writes are deferred past mf=3's matmuls so the
            # input stream gets clean HBM bandwidth; final mf runs in two
            # rg-halves so its copies/writes stream instead of piling up.
            deferred = []

            def emit_write(y_sb, mf, rg):
                eng = nc.scalar if (mf + rg) % 2 == 0 else nc.gpsimd
                eng.dma_start(
                    out=yt_out[mf * P : (mf + 1) * P, rg * FH : (rg + 1) * FH],
                    in_=y_sb,
                )

            for mf in range(KB):
                ps = [
                    psum.tile([P, FH], F32, tag="ps", name=f"ps_{mf}_{rg}")
                    for rg in range(NRG)
                ]
                # the last mf runs as two rg-halves so the first half's
                # copies/writes stream during the second half's matmuls
                rg_groups = (
                    [range(0, 4), range(4, 7), range(7, NRG)]
                    if mf == KB - 1
                    else [range(NRG)]
                )
                for rgs in rg_groups:
                    for kb in range(mf + 1):
                        w_st = wtp[:, WT_IDX[(kb, mf)], :]
                        for rg in rgs:
                            nc.tensor.matmul(
                                ps[rg],
                                w_st,
                                xtall[:, kb, rg * FH : (rg + 1) * FH],
                                start=(kb == 0),
                                stop=(kb == mf),
                            )
                if mf == 3:
                    # guard: scalar waits for the last read chunk before
                    # issuing the deferred writes, keeping HBM bandwidth
                    # clean for the input stream
                    guard = yout.tile([P, FH], F16, tag="y", name="guard")
                    nc.gpsimd.tensor_copy(guard[:, 0:8], xtall[:, KB - 1, 4088:4096])
                    for y_sb, dmf, drg in deferred:
                        nc.gpsimd.dma_start(
                            out=yt_out[
                                dmf * P : (dmf + 1) * P,
                                drg * FH : (drg + 1) * FH,
                            ],
                            in_=y_sb,
                        )
                    deferred = []
                for rg in range(NRG):
                    y_sb = yout.tile([P, FH], F16, tag="y", name=f"y_{mf}_{rg}")
                    if rg % 2 == 0:
                        nc.vector.tensor_copy(y_sb, ps[rg])
                    else:
                        nc.scalar.copy(y_sb, ps[rg])
                    if mf <= 2:
                        deferred.append((y_sb, mf, rg))
                    else:
                        emit_write(y_sb, mf, rg)

    nc.compile()
    return nc


_programs: dict = {}


def _get_programs():
    if "p1" not in _programs:
        _programs["p1"] = (
            build_phase1_fp8dr() if GRAM_MODE == "fp8dr" else build_phase1_fp16()
        )
        _programs["p2"] = build_phase2()
    return _programs["p1"], _programs["p2"]


def kernel(X, running_mean, running_cov, beta, trace=False):
    X = np.asarray(X, dtype=np.float32)
    beta = np.asarray(beta, dtype=np.float32)
    assert X.shape == (N_TOTAL, F)

    p1, p2 = _get_programs()
    core_ids = list(range(N_CORES))

    p1_dt = ml_dtypes.float8_e4m3 if GRAM_MODE == "fp8dr" else np.float16
    shards1 = X.astype(p1_dt).reshape(N_CORES, NC_ROWS, F)

    tkw = {"trace_cores": core_ids} if trace else {}

    def _run(prog, in_maps):
        # transient NRT/device hiccups (incl. NRT_EXEC_UNIT_UNRECOVERABLE
        # wedges) have been observed; back off and retry
        import time as _time

        for attempt, pause in enumerate((2.0, 10.0, 30.0)):
            try:
                return run_bass_kernel_spmd(
                    prog, in_maps, core_ids, trace=trace, **tkw
                )
            except Exception:
                _time.sleep(pause)
        return run_bass_kernel_spmd(prog, in_maps, core_ids, trace=trace, **tkw)

    in1 = [{"x": np.ascontiguousarray(shards1[i])} for i in range(N_CORES)]
    r1 = _run(p1, in1)
    kernel.exec_ns_phase1 = r1.exec_time_ns

    gram = np.zeros((F, F), dtype=np.float64)
    colsum = np.zeros((F,), dtype=np.float64)
    for res in r1.results:
        gram += res["gram"].astype(np.float64)
        colsum += res["colsum"].astype(np.float64).sum(axis=0)
    # mirror the computed lower triangle onto the upper
    gram = np.tril(gram) + np.tril(gram, -1).T

    mean = colsum / N_TOTAL
    cov = gram / N_TOTAL - np.outer(mean, mean)
    a = cov + EPS * np.eye(F, dtype=np.float64)
    L = np.linalg.cholesky(a)
    w = np.linalg.solve(L, np.eye(F, dtype=np.float64))  # W = L^-1
    wt = w.T  # upper triangular [k, f]
    wtp = np.zeros((P, len(WT_BLOCKS), P), dtype=np.float16)
    for (kb, mf), i in WT_IDX.items():
        wtp[:, i, :] = wt[kb * P : (kb + 1) * P, mf * P : (mf + 1) * P].astype(
            np.float16
        )
    b = (beta.astype(np.float64) - w @ mean).astype(np.float32)

    xts = np.ascontiguousarray(
        X.astype(np.float16).reshape(N_CORES, NC_ROWS, F).transpose(0, 2, 1)
    )
    in2 = [{"xt": xts[i], "wtp": wtp} for i in range(N_CORES)]
    r2 = _run(p2, in2)
    kernel.exec_ns_phase2 = r2.exec_time_ns

    # host epilogue: bias + upcast + transpose back (O(N F))
    y = np.empty((N_TOTAL, F), dtype=np.float32)
    for i, res in enumerate(r2.results):
        y[i * NC_ROWS : (i + 1) * NC_ROWS, :] = (
            res["yt"].astype(np.float32) + b[:, None]
        ).T
    return y


kernel.exec_ns_phase1 = None
kernel.exec_ns_phase2 = None


# revision 41
# speedup vs baseline: 1.0299x; 1.0019x over previous
"""Batch whitening (Cholesky) kernel for Trainium2, 8 NeuronCores.

Computes, for X [32768, 1024] (matching the reference nn_BWCholeskyBlock):
    mean = X.mean(0); xc = X - mean; cov = xc.T @ xc / N
    L = chol(cov + eps I);  Y = (L^-1 xc^T).T + beta

Strategy (data-parallel over batch, 8 cores; harness gate rel_err < 2e-2):
  Phase 1 (device): per-core partial gram  G_i = Xq_i^T Xq_i  and column
     sums.  Two dtype modes:
       fp16:  [128,256] gram tiles                   (~72us MM stream)
       fp8dr: e4m3 DoubleRow [128,512] gram tiles, K=256 per matmul --
              runs at the fp8 PE peak               (~44us MM stream)
     Column sums ride on VectorE (wide [P,2,F] fp32 accumulators),
     fully hidden under the matmul stream.
  Host: reduce partials, mirror the triangle -> mean, cov; Cholesky +
     triangular inverse of the small [F,F] factor (replicated per the
     sharding hint).
  Phase 2 (device): per-core  Yt_i = W @ Xq_i^T  (fp16) as 288 N=512
     matmuls walking the lower triangle of W at 128-granularity; the
     stationary [128,128] W^T block is reused across 8 row-group matmuls.
     Yt written back as fp16 (halves write traffic); host adds
     b = beta - W mean, upcasts, transposes (O(N F) epilogue only).
"""
import sys

sys.path.insert(0, "/opt/trn_rl_repo")

import numpy as np
import ml_dtypes

import concourse.bass as bass
import concourse.mybir as mybir
import concourse.tile as tile
from concourse import bacc
from concourse.bass_utils import run_bass_kernel_spmd

EPS = 1e-5
N_CORES = 8
N_TOTAL = 32768
F = 1024
NC_ROWS = N_TOTAL // N_CORES  # 4096 rows per core
NT = NC_ROWS // 128           # 32 row-tiles per core
NG2 = NC_ROWS // 256          # 16 super-tiles (fp8 DoubleRow, K=256)
P = 128
FH = F // 2                   # 512
FQ = F // 4                   # 256
KB = F // P                   # 8 column blocks of 128

F32 = mybir.dt.float32
F16 = mybir.dt.float16
F8 = mybir.dt.float8e4
DR = mybir.MatmulPerfMode.DoubleRow

GRAM_MODE = "fp8dr"  # "fp16" | "fp8dr"

# fp16 gram tiles (mf, nq): rows mf*128..+128, cols nq*256..+256, ordered
# by mf; pass A = mf 0..6 (16 tiles = 8 PSUM banks x 2 halves), B = mf 7.
GRAM_TILES = sorted([(mf, nq) for nq in range(4) for mf in range(2 * nq, KB)])
GRAM_A = [t for t in GRAM_TILES if t[0] < 7]
GRAM_B = [t for t in GRAM_TILES if t[0] == 7]

# fp8dr gram tiles (mf, nh): rows mf*128..+128, cols nh*512..+512
DR_A = [(mf, 0) for mf in range(KB)]       # left half, 8 banks
DR_B1 = [(4, 1), (5, 1)]                   # lower-right quarter in three
DR_B2 = [(6, 1)]                           # passes so copies stream and
DR_B3 = [(7, 1)]                           # the final tail is one tile


def build_phase1_fp16() -> bass.Bass:
    nc = bacc.Bacc(None, target_bir_lowering=False, debug=False)

    x_in = nc.dram_tensor("x", [NC_ROWS, F], F16, kind="ExternalInput")
    gram_out = nc.dram_tensor("gram", [F, F], F32, kind="ExternalOutput")
    colsum_out = nc.dram_tensor("colsum", [P, F], F32, kind="ExternalOutput")

    with tile.TileContext(nc) as tc:
        with (
            tc.tile_pool(name="xres", bufs=1) as xres,
            tc.tile_pool(name="work", bufs=1) as work,
            tc.tile_pool(name="gout", bufs=8) as gout,
            tc.tile_pool(name="psum", bufs=8, space="PSUM") as psum,
        ):
            xt = []
            for nt in range(NT):
                t = xres.tile([P, F], F16, tag=f"x{nt}")
                if nt == 0:
                    nc.sync.dma_start(out=t[:, 0:FH], in_=x_in[0:P, 0:FH])
                    nc.sync.dma_start(out=t[:, FH:F], in_=x_in[0:P, FH:F])
                else:
                    eng = nc.sync if nt % 2 == 0 else nc.scalar
                    eng.dma_start(out=t, in_=x_in[nt * P : (nt + 1) * P, :])
                xt.append(t)

            acc = []
            for j in range(4):
                a = work.tile([P, F], F32, tag=f"acc{j}")
                nc.vector.memset(a, 0.0)
                acc.append(a)
            for nt in range(NT):
                nc.vector.tensor_add(acc[nt % 4], acc[nt % 4], xt[nt])
            nc.vector.tensor_add(acc[0], acc[0], acc[1])
            nc.vector.tensor_add(acc[2], acc[2], acc[3])
            nc.vector.tensor_add(acc[0], acc[0], acc[2])
            nc.sync.dma_start(out=colsum_out[:, :], in_=acc[0])

            for pi, tiles in enumerate([GRAM_A, GRAM_B]):
                npair = (len(tiles) + 1) // 2
                ps = [
                    psum.tile([P, 2, FQ], F32, tag="g", name=f"g_{pi}_{i}")
                    for i in range(npair)
                ]
                for nt in range(NT):
                    for i, (mf, nq) in enumerate(tiles):
                        nc.tensor.matmul(
                            ps[i % npair][:, i // npair, :],
                            xt[nt][:, mf * P : (mf + 1) * P],
                            xt[nt][:, nq * FQ : (nq + 1) * FQ],
                            start=(nt == 0 and i < npair),
                            stop=(nt == NT - 1),
                        )
                for j in range(npair):
                    for h in range(2):
                        i = j + h * npair
                        if i >= len(tiles):
                            continue
                        mf, nq = tiles[i]
                        g_sb = gout.tile(
                            [P, FQ], F32, tag="gsb", name=f"gsb_{mf}_{nq}"
                        )
                        if h == 0:
                            nc.scalar.copy(g_sb, ps[j][:, h, :])
                        else:
                            nc.vector.tensor_copy(g_sb, ps[j][:, h, :])
                        eng = nc.sync if (j + h) % 2 == 0 else nc.scalar
                        eng.dma_start(
                            out=gram_out[
                                mf * P : (mf + 1) * P, nq * FQ : (nq + 1) * FQ
                            ],
                            in_=g_sb,
                        )

    nc.compile()
    return nc


def build_phase1_fp8dr() -> bass.Bass:
    """e4m3 DoubleRow gram: K=256 per matmul, [128,512] tiles; colsum via
    ones-stationary matmuls (keeps VectorE off the critical path)."""
    nc = bacc.Bacc(None, target_bir_lowering=False, debug=False)

    x_in = nc.dram_tensor("x", [NC_ROWS, F], F8, kind="ExternalInput")
    gram_out = nc.dram_tensor("gram", [F, F], F32, kind="ExternalOutput")
    colsum_out = nc.dram_tensor("colsum", [P, F], F32, kind="ExternalOutput")

    # super-tile g holds rows g*256..(g+1)*256 as [pi, po, f], row=g*256+po*128+pi
    x_r = x_in.rearrange("(g po p) f -> p g po f", p=P, po=2)

    with tile.TileContext(nc) as tc:
        with (
            tc.tile_pool(name="xres", bufs=1) as xres,
            tc.tile_pool(name="work", bufs=1) as work,
            tc.tile_pool(name="gout", bufs=8) as gout,
            tc.tile_pool(name="psum", bufs=8, space="PSUM") as psum,
        ):
            # short HAM warmup on a memset tile (no DMA dependency): ~3.4us
            # of PE busy ending as the first x chunk lands, so the DR
            # stream starts at full clock
            ww = work.tile([P, FH], F8)
            nc.vector.memset(ww, 0.125)
            wu = psum.tile([P, FH], F32, tag="g", name="wu")
            NWU = 8
            for i in range(NWU):
                nc.tensor.matmul(
                    wu,
                    ww[:, 0:P],
                    ww,
                    start=(i == 0),
                    stop=(i == NWU - 1),
                )
            wu_sb = gout.tile([P, FH], F32, tag="gsb", name="wu_sb")
            nc.scalar.copy(wu_sb[:, 0:P], wu[:, 0:P])

            # colsum on VectorE with two wide [P,2,F] accumulators (few,
            # big DVE ops -- per-instruction overhead dominates small adds);
            # hidden under the LDW-gated matmul stream
            acc = []
            for j in range(2):
                a = work.tile([P, 2, F], F32, tag=f"acc{j}")
                nc.vector.memset(a, 0.0)
                acc.append(a)
            csum = work.tile([P, F], F32)

            xs = []
            for g in range(NG2):
                t = xres.tile([P, 2, F], F8, tag=f"x{g}")
                if g == 0:
                    # first matmul touches only cols 0:512 of both halves
                    nc.sync.dma_start(out=t[:, :, 0:FH], in_=x_r[:, 0, :, 0:FH])
                    nc.sync.dma_start(out=t[:, :, FH:F], in_=x_r[:, 0, :, FH:F])
                else:
                    nc.sync.dma_start(out=t, in_=x_r[:, g])
                xs.append(t)
                nc.vector.tensor_add(acc[g % 2], acc[g % 2], t)
            nc.vector.tensor_add(acc[0], acc[0], acc[1])
            nc.vector.tensor_add(csum, acc[0][:, 0, :], acc[0][:, 1, :])
            nc.sync.dma_start(out=colsum_out[:, :], in_=csum)

            for pi, tiles in enumerate([DR_A, DR_B1, DR_B2, DR_B3]):
                ps = [
                    psum.tile([P, FH], F32, tag="g", name=f"g_{pi}_{i}")
                    for i in range(len(tiles))
                ]
                for g in range(NG2):
                    for i, (mf, nh) in enumerate(tiles):
                        nc.tensor.matmul(
                            ps[i],
                            xs[g][:, :, mf * P : (mf + 1) * P],
                            xs[g][:, :, nh * FH : (nh + 1) * FH],
                            start=(g == 0),
                            stop=(g == NG2 - 1),
                            perf_mode=DR,
                        )
                for i, (mf, nh) in enumerate(tiles):
                    g_sb = gout.tile([P, FH], F32, tag="gsb", name=f"gsb_{mf}_{nh}")
                    nc.scalar.copy(g_sb, ps[i])
                    eng = nc.gpsimd if i % 2 == 0 else nc.scalar
                    eng.dma_start(
                        out=gram_out[
                            mf * P : (mf + 1) * P, nh * FH : (nh + 1) * FH
                        ],
                        in_=g_sb,
                    )

    nc.compile()
    return nc


# phase-2 W^T stationary blocks: (kb, mf) with kb <= mf (W lower tri)
WT_BLOCKS = [(kb, mf) for mf in range(KB) for kb in range(mf + 1)]
WT_IDX = {bm: i for i, bm in enumerate(WT_BLOCKS)}
NRG = NC_ROWS // FH  # 8 row groups of 512


def build_phase2() -> bass.Bass:
    """Per-core: yt [F, NC_ROWS] (fp16) = W @ Xq^T  (fp16 in, no bias)."""
    nc = bacc.Bacc(None, target_bir_lowering=False, debug=False)

    xt_in = nc.dram_tensor("xt", [F, NC_ROWS], F16, kind="ExternalInput")
    wtp_in = nc.dram_tensor(
        "wtp", [P, len(WT_BLOCKS), P], F16, kind="ExternalInput"
    )
    yt_out = nc.dram_tensor("yt", [F, NC_ROWS], F16, kind="ExternalOutput")

    xt_r = xt_in.rearrange("(kb p) n -> p kb n", p=P)  # [128, 8, NC_ROWS]

    with tile.TileContext(nc) as tc:
        with (
            tc.tile_pool(name="singles", bufs=1) as singles,
            tc.tile_pool(name="yout", bufs=52) as yout,
            tc.tile_pool(name="psum", bufs=8, space="PSUM") as psum,
        ):
            xtall = singles.tile([P, KB, NC_ROWS], F16)
            wtp = singles.tile([P, len(WT_BLOCKS), P], F16)
            ww = singles.tile([P, P], F16)

            # HAM warmup: matmuls on a memset tile start right after the
            # preamble (no DMA dependency), so the PE reaches full clock
            # before the real stream and the early DMA waits are absorbed
            nc.vector.memset(ww, 0.125)
            wu = psum.tile([P, FH], F32, tag="ps", name="wu")
            NWU = 56
            for i in range(NWU):
                nc.tensor.matmul(
                    wu[:, 0:P],
                    ww,
                    ww,
                    start=(i == 0),
                    stop=(i == NWU - 1),
                )
            wu_sb = yout.tile([P, FH], F16, tag="y", name="wu_sb")
            nc.vector.tensor_copy(wu_sb[:, 0:P], wu[:, 0:P])

            # all reads share the sync ring, which drains FIFO -- exact
            # need-order priority: wtp blocks for phase mf land just
            # before xt chunk kb=mf (both gate phase mf); early chunks
            # split so matmuls unblock sooner
            def wtp_load(mf):
                lo, hi = mf * (mf + 1) // 2, (mf + 1) * (mf + 2) // 2
                nc.sync.dma_start(out=wtp[:, lo:hi, :], in_=wtp_in[:, lo:hi, :])

            wtp_load(0)
            for c0, c1 in [(0, 512), (512, 1024), (1024, 2048), (2048, 4096)]:
                nc.sync.dma_start(out=xtall[:, 0, c0:c1], in_=xt_r[:, 0, c0:c1])
            for kb in (1, 2):
                wtp_load(kb)
                for h in range(2):
                    nc.sync.dma_start(
                        out=xtall[:, kb, h * 2048 : (h + 1) * 2048],
                        in_=xt_r[:, kb, h * 2048 : (h + 1) * 2048],
                    )
            for kb in range(3, KB):
                wtp_load(kb)
                nc.sync.dma_start(out=xtall[:, kb, :], in_=xt_r[:, kb, :])

            # triangular apply: output f-block mf accumulates over kb<=mf;
            # stationary W^T block reused across the 8 row-group matmuls.
            # Early phases' writes are deferred past mf=3's matmuls so the
            # input stream gets clean HBM bandwidth; final mf runs in two
            # rg-halves so its copies/writes stream instead of piling up.
            deferred = []

            def emit_write(y_sb, mf, rg):
                eng = nc.scalar if (mf + rg) % 2 == 0 else nc.gpsimd
                eng.dma_start(
                    out=yt_out[mf * P : (mf + 1) * P, rg * FH : (rg + 1) * FH],
                    in_=y_sb,
                )

            for mf in range(KB):
                ps = [
                    psum.tile([P, FH], F32, tag="ps", name=f"ps_{mf}_{rg}")
                    for rg in range(NRG)
                ]
                # the last mf runs as two rg-halves so the first half's
                # copies/writes stream during the second half's matmuls
                rg_groups = (
                    [range(0, 4), range(4, 7), range(7, NRG)]
                    if mf == KB - 1
                    else [range(NRG)]
                )
                for rgs in rg_groups:
                    for kb in range(mf + 1):
                        w_st = wtp[:, WT_IDX[(kb, mf)], :]
                        for rg in rgs:
                            nc.tensor.matmul(
                                ps[rg],
                                w_st,
                                xtall[:, kb, rg * FH : (rg + 1) * FH],
                                start=(kb == 0),
                                stop=(kb == mf),
                            )
                if mf == 5:
                    # guard: gpsimd waits for the last read chunk before
                    # issuing the deferred writes, keeping HBM bandwidth
                    # clean for the whole input stream
                    guard = yout.tile([P, FH], F16, tag="y", name="guard")
                    nc.gpsimd.tensor_copy(guard[:, 0:8], xtall[:, KB - 1, 4088:4096])
                    for y_sb, dmf, drg in deferred:
                        nc.gpsimd.dma_start(
                            out=yt_out[
                                dmf * P : (dmf + 1) * P,
                                drg * FH : (drg + 1) * FH,
                            ],
                            in_=y_sb,
                        )
                    deferred = []
                for rg in range(NRG):
                    y_sb = yout.tile([P, FH], F16, tag="y", name=f"y_{mf}_{rg}")
                    if rg % 2 == 0:
                        nc.vector.tensor_copy(y_sb, ps[rg])
                    else:
                        nc.scalar.copy(y_sb, ps[rg])
                    if mf <= 4:
                        deferred.append((y_sb, mf, rg))
                    else:
                        emit_write(y_sb, mf, rg)

    nc.compile()
    return nc


_programs: dict = {}


def _get_programs():
    if "p1" not in _programs:
        _programs["p1"] = (
            build_phase1_fp8dr() if GRAM_MODE == "fp8dr" else build_phase1_fp16()
        )
        _programs["p2"] = build_phase2()
    return _programs["p1"], _programs["p2"]


def kernel(X, running_mean, running_cov, beta, trace=False):
    X = np.asarray(X, dtype=np.float32)
    beta = np.asarray(beta, dtype=np.float32)
    assert X.shape == (N_TOTAL, F)

    p1, p2 = _get_programs()
    core_ids = list(range(N_CORES))

    p1_dt = ml_dtypes.float8_e4m3 if GRAM_MODE == "fp8dr" else np.float16
    shards1 = X.astype(p1_dt).reshape(N_CORES, NC_ROWS, F)

    tkw = {"trace_cores": core_ids} if trace else {}

    def _run(prog, in_maps):
        # transient NRT/device hiccups (incl. NRT_EXEC_UNIT_UNRECOVERABLE
        # wedges) have been observed; back off and retry
        import time as _time

        for attempt, pause in enumerate((2.0, 10.0, 30.0)):
            try:
                return run_bass_kernel_spmd(
                    prog, in_maps, core_ids, trace=trace, **tkw
                )
            except Exception:
                _time.sleep(pause)
        return run_bass_kernel_spmd(prog, in_maps, core_ids, trace=trace, **tkw)

    in1 = [{"x": np.ascontiguousarray(shards1[i])} for i in range(N_CORES)]
    r1 = _run(p1, in1)
    kernel.exec_ns_phase1 = r1.exec_time_ns

    gram = np.zeros((F, F), dtype=np.float64)
    colsum = np.zeros((F,), dtype=np.float64)
    for res in r1.results:
        gram += res["gram"].astype(np.float64)
        colsum += res["colsum"].astype(np.float64).sum(axis=0)
    # mirror the computed lower triangle onto the upper
    gram = np.tril(gram) + np.tril(gram, -1).T

    mean = colsum / N_TOTAL
    cov = gram / N_TOTAL - np.outer(mean, mean)
    a = cov + EPS * np.eye(F, dtype=np.float64)
    L = np.linalg.cholesky(a)
    w = np.linalg.solve(L, np.eye(F, dtype=np.float64))  # W = L^-1
    wt = w.T  # upper triangular [k, f]
    wtp = np.zeros((P, len(WT_BLOCKS), P), dtype=np.float16)
    for (kb, mf), i in WT_IDX.items():
        wtp[:, i, :] = wt[kb * P : (kb + 1) * P, mf * P : (mf + 1) * P].astype(
            np.float16
        )
    b = (beta.astype(np.float64) - w @ mean).astype(np.float32)

    xts = np.ascontiguousarray(
        X.astype(np.float16).reshape(N_CORES, NC_ROWS, F).transpose(0, 2, 1)
    )
    in2 = [{"xt": xts[i], "wtp": wtp} for i in range(N_CORES)]
    r2 = _run(p2, in2)
    kernel.exec_ns_phase2 = r2.exec_time_ns

    # host epilogue: bias + upcast + transpose back (O(N F))
    y = np.empty((N_TOTAL, F), dtype=np.float32)
    for i, res in enumerate(r2.results):
        y[i * NC_ROWS : (i + 1) * NC_ROWS, :] = (
            res["yt"].astype(np.float32) + b[:, None]
        ).T
    return y


kernel.exec_ns_phase1 = None
kernel.exec_ns_phase2 = None


# revision 42
# speedup vs baseline: 1.0699x; 1.0389x over previous
"""Batch whitening (Cholesky) kernel for Trainium2, 8 NeuronCores.

Computes, for X [32768, 1024] (matching the reference nn_BWCholeskyBlock):
    mean = X.mean(0); xc = X - mean; cov = xc.T @ xc / N
    L = chol(cov + eps I);  Y = (L^-1 xc^T).T + beta

Strategy (data-parallel over batch, 8 cores; harness gate rel_err < 2e-2):
  Phase 1 (device): per-core partial gram  G_i = Xq_i^T Xq_i  and column
     sums.  Two dtype modes:
       fp16:  [128,256] gram tiles                   (~72us MM stream)
       fp8dr: e4m3 DoubleRow [128,512] gram tiles, K=256 per matmul --
              runs at the fp8 PE peak               (~44us MM stream)
     Column sums ride on VectorE (wide [P,2,F] fp32 accumulators),
     fully hidden under the matmul stream.
  Host: reduce partials, mirror the triangle -> mean, cov; Cholesky +
     triangular inverse of the small [F,F] factor (replicated per the
     sharding hint).
  Phase 2 (device): per-core  Yt_i = W @ Xq_i^T  (fp16) as 288 N=512
     matmuls walking the lower triangle of W at 128-granularity; the
     stationary [128,128] W^T block is reused across 8 row-group matmuls.
     Yt written back as fp16 (halves write traffic); host adds
     b = beta - W mean, upcasts, transposes (O(N F) epilogue only).
"""
import sys

sys.path.insert(0, "/opt/trn_rl_repo")

import numpy as np
import ml_dtypes

import concourse.bass as bass
import concourse.mybir as mybir
import concourse.tile as tile
from concourse import bacc
from concourse.bass_utils import run_bass_kernel_spmd

EPS = 1e-5
N_CORES = 8
N_TOTAL = 32768
F = 1024
NC_ROWS = N_TOTAL // N_CORES  # 4096 rows per core
NT = NC_ROWS // 128           # 32 row-tiles per core
NG2 = NC_ROWS // 256          # 16 super-tiles (fp8 DoubleRow, K=256)
P = 128
FH = F // 2                   # 512
FQ = F // 4                   # 256
KB = F // P                   # 8 column blocks of 128

F32 = mybir.dt.float32
F16 = mybir.dt.float16
F8 = mybir.dt.float8e4
DR = mybir.MatmulPerfMode.DoubleRow

GRAM_MODE = "fp8dr"  # "fp16" | "fp8dr"

# fp16 gram tiles (mf, nq): rows mf*128..+128, cols nq*256..+256, ordered
# by mf; pass A = mf 0..6 (16 tiles = 8 PSUM banks x 2 halves), B = mf 7.
GRAM_TILES = sorted([(mf, nq) for nq in range(4) for mf in range(2 * nq, KB)])
GRAM_A = [t for t in GRAM_TILES if t[0] < 7]
GRAM_B = [t for t in GRAM_TILES if t[0] == 7]

# fp8dr gram tiles (mf, nh): rows mf*128..+128, cols nh*512..+512
DR_A = [(mf, 0) for mf in range(KB)]       # left half, 8 banks
DR_B1 = [(4, 1), (5, 1)]                   # lower-right quarter in three
DR_B2 = [(6, 1)]                           # passes so copies stream and
DR_B3 = [(7, 1)]                           # the final tail is one tile


def build_phase1_fp16() -> bass.Bass:
    nc = bacc.Bacc(None, target_bir_lowering=False, debug=False)

    x_in = nc.dram_tensor("x", [NC_ROWS, F], F16, kind="ExternalInput")
    gram_out = nc.dram_tensor("gram", [F, F], F32, kind="ExternalOutput")
    colsum_out = nc.dram_tensor("colsum", [P, F], F32, kind="ExternalOutput")

    with tile.TileContext(nc) as tc:
        with (
            tc.tile_pool(name="xres", bufs=1) as xres,
            tc.tile_pool(name="work", bufs=1) as work,
            tc.tile_pool(name="gout", bufs=8) as gout,
            tc.tile_pool(name="psum", bufs=8, space="PSUM") as psum,
        ):
            xt = []
            for nt in range(NT):
                t = xres.tile([P, F], F16, tag=f"x{nt}")
                if nt == 0:
                    nc.sync.dma_start(out=t[:, 0:FH], in_=x_in[0:P, 0:FH])
                    nc.sync.dma_start(out=t[:, FH:F], in_=x_in[0:P, FH:F])
                else:
                    eng = nc.sync if nt % 2 == 0 else nc.scalar
                    eng.dma_start(out=t, in_=x_in[nt * P : (nt + 1) * P, :])
                xt.append(t)

            acc = []
            for j in range(4):
                a = work.tile([P, F], F32, tag=f"acc{j}")
                nc.vector.memset(a, 0.0)
                acc.append(a)
            for nt in range(NT):
                nc.vector.tensor_add(acc[nt % 4], acc[nt % 4], xt[nt])
            nc.vector.tensor_add(acc[0], acc[0], acc[1])
            nc.vector.tensor_add(acc[2], acc[2], acc[3])
            nc.vector.tensor_add(acc[0], acc[0], acc[2])
            nc.sync.dma_start(out=colsum_out[:, :], in_=acc[0])

            for pi, tiles in enumerate([GRAM_A, GRAM_B]):
                npair = (len(tiles) + 1) // 2
                ps = [
                    psum.tile([P, 2, FQ], F32, tag="g", name=f"g_{pi}_{i}")
                    for i in range(npair)
                ]
                for nt in range(NT):
                    for i, (mf, nq) in enumerate(tiles):
                        nc.tensor.matmul(
                            ps[i % npair][:, i // npair, :],
                            xt[nt][:, mf * P : (mf + 1) * P],
                            xt[nt][:, nq * FQ : (nq + 1) * FQ],
                            start=(nt == 0 and i < npair),
                            stop=(nt == NT - 1),
                        )
                for j in range(npair):
                    for h in range(2):
                        i = j + h * npair
                        if i >= len(tiles):
                            continue
                        mf, nq = tiles[i]
                        g_sb = gout.tile(
                            [P, FQ], F32, tag="gsb", name=f"gsb_{mf}_{nq}"
                        )
                        if h == 0:
                            nc.scalar.copy(g_sb, ps[j][:, h, :])
                        else:
                            nc.vector.tensor_copy(g_sb, ps[j][:, h, :])
                        eng = nc.sync if (j + h) % 2 == 0 else nc.scalar
                        eng.dma_start(
                            out=gram_out[
                                mf * P : (mf + 1) * P, nq * FQ : (nq + 1) * FQ
                            ],
                            in_=g_sb,
                        )

    nc.compile()
    return nc


def build_phase1_fp8dr() -> bass.Bass:
    """e4m3 DoubleRow gram: K=256 per matmul, [128,512] tiles; colsum via
    ones-stationary matmuls (keeps VectorE off the critical path)."""
    nc = bacc.Bacc(None, target_bir_lowering=False, debug=False)

    x_in = nc.dram_tensor("x", [NC_ROWS, F], F8, kind="ExternalInput")
    gram_out = nc.dram_tensor("gram", [F, F], F32, kind="ExternalOutput")
    colsum_out = nc.dram_tensor("colsum", [P, F], F32, kind="ExternalOutput")

    # super-tile g holds rows g*256..(g+1)*256 as [pi, po, f], row=g*256+po*128+pi
    x_r = x_in.rearrange("(g po p) f -> p g po f", p=P, po=2)

    with tile.TileContext(nc) as tc:
        with (
            tc.tile_pool(name="xres", bufs=1) as xres,
            tc.tile_pool(name="work", bufs=1) as work,
            tc.tile_pool(name="gout", bufs=8) as gout,
            tc.tile_pool(name="psum", bufs=8, space="PSUM") as psum,
        ):
            # short HAM warmup on a memset tile (no DMA dependency): ~3.4us
            # of PE busy ending as the first x chunk lands, so the DR
            # stream starts at full clock
            ww = work.tile([P, FH], F8)
            nc.vector.memset(ww, 0.125)
            wu = psum.tile([P, FH], F32, tag="g", name="wu")
            NWU = 8
            for i in range(NWU):
                nc.tensor.matmul(
                    wu,
                    ww[:, 0:P],
                    ww,
                    start=(i == 0),
                    stop=(i == NWU - 1),
                )
            wu_sb = gout.tile([P, FH], F32, tag="gsb", name="wu_sb")
            nc.scalar.copy(wu_sb[:, 0:P], wu[:, 0:P])

            # colsum on VectorE with two wide [P,2,F] accumulators (few,
            # big DVE ops -- per-instruction overhead dominates small adds);
            # hidden under the LDW-gated matmul stream
            acc = []
            for j in range(2):
                a = work.tile([P, 2, F], F32, tag=f"acc{j}")
                nc.vector.memset(a, 0.0)
                acc.append(a)
            csum = work.tile([P, F], F32)

            xs = []
            for g in range(NG2):
                t = xres.tile([P, 2, F], F8, tag=f"x{g}")
                if g == 0:
                    # first matmul touches only cols 0:512 of both halves
                    nc.sync.dma_start(out=t[:, :, 0:FH], in_=x_r[:, 0, :, 0:FH])
                    nc.sync.dma_start(out=t[:, :, FH:F], in_=x_r[:, 0, :, FH:F])
                else:
                    nc.sync.dma_start(out=t, in_=x_r[:, g])
                xs.append(t)
                nc.vector.tensor_add(acc[g % 2], acc[g % 2], t)
            nc.vector.tensor_add(acc[0], acc[0], acc[1])
            nc.vector.tensor_add(csum, acc[0][:, 0, :], acc[0][:, 1, :])
            nc.sync.dma_start(out=colsum_out[:, :], in_=csum)

            for pi, tiles in enumerate([DR_A, DR_B1, DR_B2, DR_B3]):
                ps = [
                    psum.tile([P, FH], F32, tag="g", name=f"g_{pi}_{i}")
                    for i in range(len(tiles))
                ]
                for g in range(NG2):
                    for i, (mf, nh) in enumerate(tiles):
                        nc.tensor.matmul(
                            ps[i],
                            xs[g][:, :, mf * P : (mf + 1) * P],
                            xs[g][:, :, nh * FH : (nh + 1) * FH],
                            start=(g == 0),
                            stop=(g == NG2 - 1),
                            perf_mode=DR,
                        )
                for i, (mf, nh) in enumerate(tiles):
                    g_sb = gout.tile([P, FH], F32, tag="gsb", name=f"gsb_{mf}_{nh}")
                    nc.scalar.copy(g_sb, ps[i])
                    eng = nc.gpsimd if i % 2 == 0 else nc.scalar
                    eng.dma_start(
                        out=gram_out[
                            mf * P : (mf + 1) * P, nh * FH : (nh + 1) * FH
                        ],
                        in_=g_sb,
                    )

    nc.compile()
    return nc


# phase-2 W^T stationary blocks: (kb, mf) with kb <= mf (W lower tri)
WT_BLOCKS = [(kb, mf) for mf in range(KB) for kb in range(mf + 1)]
WT_IDX = {bm: i for i, bm in enumerate(WT_BLOCKS)}
NRG = NC_ROWS // FH  # 8 row groups of 512


def build_phase2() -> bass.Bass:
    """Per-core: yt [F, NC_ROWS] (fp16) = W @ Xq^T  (fp16 in, no bias)."""
    nc = bacc.Bacc(None, target_bir_lowering=False, debug=False)

    xt_in = nc.dram_tensor("xt", [F, NC_ROWS], F16, kind="ExternalInput")
    wtp_in = nc.dram_tensor(
        "wtp", [P, len(WT_BLOCKS), P], F16, kind="ExternalInput"
    )
    yt_out = nc.dram_tensor("yt", [F, NC_ROWS], F16, kind="ExternalOutput")

    xt_r = xt_in.rearrange("(kb p) n -> p kb n", p=P)  # [128, 8, NC_ROWS]

    with tile.TileContext(nc) as tc:
        with (
            tc.tile_pool(name="singles", bufs=1) as singles,
            tc.tile_pool(name="yout", bufs=52) as yout,
            tc.tile_pool(name="psum", bufs=8, space="PSUM") as psum,
        ):
            xtall = singles.tile([P, KB, NC_ROWS], F16)
            wtp = singles.tile([P, len(WT_BLOCKS), P], F16)
            ww = singles.tile([P, P], F16)

            # HAM warmup: matmuls on a memset tile start right after the
            # preamble (no DMA dependency), so the PE reaches full clock
            # before the real stream and the early DMA waits are absorbed
            nc.vector.memset(ww, 0.125)
            wu = psum.tile([P, FH], F32, tag="ps", name="wu")
            NWU = 56
            for i in range(NWU):
                nc.tensor.matmul(
                    wu[:, 0:P],
                    ww,
                    ww,
                    start=(i == 0),
                    stop=(i == NWU - 1),
                )
            wu_sb = yout.tile([P, FH], F16, tag="y", name="wu_sb")
            nc.vector.tensor_copy(wu_sb[:, 0:P], wu[:, 0:P])

            # all reads share the sync ring, which drains FIFO -- exact
            # need-order priority: wtp blocks for phase mf land just
            # before xt chunk kb=mf (both gate phase mf); early chunks
            # split so matmuls unblock sooner
            def wtp_load(mf):
                lo, hi = mf * (mf + 1) // 2, (mf + 1) * (mf + 2) // 2
                nc.sync.dma_start(out=wtp[:, lo:hi, :], in_=wtp_in[:, lo:hi, :])

            wtp_load(0)
            for c0, c1 in [(0, 512), (512, 1024), (1024, 2048), (2048, 4096)]:
                nc.sync.dma_start(out=xtall[:, 0, c0:c1], in_=xt_r[:, 0, c0:c1])
            for kb in (1, 2):
                wtp_load(kb)
                for h in range(2):
                    nc.sync.dma_start(
                        out=xtall[:, kb, h * 2048 : (h + 1) * 2048],
                        in_=xt_r[:, kb, h * 2048 : (h + 1) * 2048],
                    )
            for kb in range(3, KB):
                wtp_load(kb)
                nc.sync.dma_start(out=xtall[:, kb, :], in_=xt_r[:, kb, :])

            # triangular apply: output f-block mf accumulates over kb<=mf;
            # stationary W^T block reused across the 8 row-group matmuls.
            # Early phases' writes are deferred past mf=3's matmuls so the
            # input stream gets clean HBM bandwidth; final mf runs in two
            # rg-halves so its copies/writes stream instead of piling up.
            deferred = []

            def emit_write(y_sb, mf, rg):
                # final phase's writes go via scalar HWDGE so the gpsimd
                # ring drains before the epilogue barrier
                eng = (
                    nc.scalar
                    if (mf == KB - 1 or (mf + rg) % 2 == 0)
                    else nc.gpsimd
                )
                eng.dma_start(
                    out=yt_out[mf * P : (mf + 1) * P, rg * FH : (rg + 1) * FH],
                    in_=y_sb,
                )

            for mf in range(KB):
                ps = [
                    psum.tile([P, FH], F32, tag="ps", name=f"ps_{mf}_{rg}")
                    for rg in range(NRG)
                ]
                # the last mf runs as two rg-halves so the first half's
                # copies/writes stream during the second half's matmuls
                rg_groups = (
                    [range(0, 4), range(4, 7), range(7, NRG)]
                    if mf == KB - 1
                    else [range(NRG)]
                )
                for rgs in rg_groups:
                    for kb in range(mf + 1):
                        w_st = wtp[:, WT_IDX[(kb, mf)], :]
                        for rg in rgs:
                            nc.tensor.matmul(
                                ps[rg],
                                w_st,
                                xtall[:, kb, rg * FH : (rg + 1) * FH],
                                start=(kb == 0),
                                stop=(kb == mf),
                            )
                if mf == 5:
                    # guard: gpsimd waits for the last read chunk before
                    # issuing the deferred writes, keeping HBM bandwidth
                    # clean for the whole input stream
                    guard = yout.tile([P, FH], F16, tag="y", name="guard")
                    nc.gpsimd.tensor_copy(guard[:, 0:8], xtall[:, KB - 1, 4088:4096])
                    for y_sb, dmf, drg in deferred:
                        nc.gpsimd.dma_start(
                            out=yt_out[
                                dmf * P : (dmf + 1) * P,
                                drg * FH : (drg + 1) * FH,
                            ],
                            in_=y_sb,
                        )
                    deferred = []
                for rg in range(NRG):
                    y_sb = yout.tile([P, FH], F16, tag="y", name=f"y_{mf}_{rg}")
                    if rg % 2 == 0:
                        nc.vector.tensor_copy(y_sb, ps[rg])
                    else:
                        nc.scalar.copy(y_sb, ps[rg])
                    if mf <= 4:
                        deferred.append((y_sb, mf, rg))
                    else:
                        emit_write(y_sb, mf, rg)

    nc.compile()
    return nc


_programs: dict = {}


def _get_programs():
    if "p1" not in _programs:
        _programs["p1"] = (
            build_phase1_fp8dr() if GRAM_MODE == "fp8dr" else build_phase1_fp16()
        )
        _programs["p2"] = build_phase2()
    return _programs["p1"], _programs["p2"]


def kernel(X, running_mean, running_cov, beta, trace=False):
    X = np.asarray(X, dtype=np.float32)
    beta = np.asarray(beta, dtype=np.float32)
    assert X.shape == (N_TOTAL, F)

    p1, p2 = _get_programs()
    core_ids = list(range(N_CORES))

    p1_dt = ml_dtypes.float8_e4m3 if GRAM_MODE == "fp8dr" else np.float16
    shards1 = X.astype(p1_dt).reshape(N_CORES, NC_ROWS, F)

    tkw = {"trace_cores": core_ids} if trace else {}

    def _run(prog, in_maps):
        # transient NRT/device hiccups (incl. NRT_EXEC_UNIT_UNRECOVERABLE
        # wedges) have been observed; back off and retry
        import time as _time

        for attempt, pause in enumerate((2.0, 10.0, 30.0)):
            try:
                return run_bass_kernel_spmd(
                    prog, in_maps, core_ids, trace=trace, **tkw
                )
            except Exception:
                _time.sleep(pause)
        return run_bass_kernel_spmd(prog, in_maps, core_ids, trace=trace, **tkw)

    in1 = [{"x": np.ascontiguousarray(shards1[i])} for i in range(N_CORES)]
    r1 = _run(p1, in1)
    kernel.exec_ns_phase1 = r1.exec_time_ns

    gram = np.zeros((F, F), dtype=np.float64)
    colsum = np.zeros((F,), dtype=np.float64)
    for res in r1.results:
        gram += res["gram"].astype(np.float64)
        colsum += res["colsum"].astype(np.float64).sum(axis=0)
    # mirror the computed lower triangle onto the upper
    gram = np.tril(gram) + np.tril(gram, -1).T

    mean = colsum / N_TOTAL
    cov = gram / N_TOTAL - np.outer(mean, mean)
    a = cov + EPS * np.eye(F, dtype=np.float64)
    L = np.linalg.cholesky(a)
    w = np.linalg.solve(L, np.eye(F, dtype=np.float64))  # W = L^-1
    wt = w.T  # upper triangular [k, f]
    wtp = np.zeros((P, len(WT_BLOCKS), P), dtype=np.float16)
    for (kb, mf), i in WT_IDX.items():
        wtp[:, i, :] = wt[kb * P : (kb + 1) * P, mf * P : (mf + 1) * P].astype(
            np.float16
        )
    b = (beta.astype(np.float64) - w @ mean).astype(np.float32)

    xts = np.ascontiguousarray(
        X.astype(np.float16).reshape(N_CORES, NC_ROWS, F).transpose(0, 2, 1)
    )
    in2 = [{"xt": xts[i], "wtp": wtp} for i in range(N_CORES)]
    r2 = _run(p2, in2)
    kernel.exec_ns_phase2 = r2.exec_time_ns

    # host epilogue: bias + upcast + transpose back (O(N F))
    y = np.empty((N_TOTAL, F), dtype=np.float32)
    for i, res in enumerate(r2.results):
        y[i * NC_ROWS : (i + 1) * NC_ROWS, :] = (
            res["yt"].astype(np.float32) + b[:, None]
        ).T
    return y


kernel.exec_ns_phase1 = None
kernel.exec_ns_phase2 = None


# revision 43
# speedup vs baseline: 1.0704x; 1.0004x over previous
"""Batch whitening (Cholesky) kernel for Trainium2, 8 NeuronCores.

Computes, for X [32768, 1024] (matching the reference nn_BWCholeskyBlock):
    mean = X.mean(0); xc = X - mean; cov = xc.T @ xc / N
    L = chol(cov + eps I);  Y = (L^-1 xc^T).T + beta

Strategy (data-parallel over batch, 8 cores; harness gate rel_err < 2e-2):
  Phase 1 (device): per-core partial gram  G_i = Xq_i^T Xq_i  and column
     sums.  Two dtype modes:
       fp16:  [128,256] gram tiles                   (~72us MM stream)
       fp8dr: e4m3 DoubleRow [128,512] gram tiles, K=256 per matmul --
              runs at the fp8 PE peak               (~44us MM stream)
     Column sums ride on VectorE (wide [P,2,F] fp32 accumulators),
     fully hidden under the matmul stream.
  Host: reduce partials, mirror the triangle -> mean, cov; Cholesky +
     triangular inverse of the small [F,F] factor (replicated per the
     sharding hint).
  Phase 2 (device): per-core  Yt_i = W @ Xq_i^T  (fp16) as 288 N=512
     matmuls walking the lower triangle of W at 128-granularity; the
     stationary [128,128] W^T block is reused across 8 row-group matmuls.
     Yt written back as fp16 (halves write traffic); host adds
     b = beta - W mean, upcasts, transposes (O(N F) epilogue only).
"""
import sys

sys.path.insert(0, "/opt/trn_rl_repo")

import numpy as np
import ml_dtypes

import concourse.bass as bass
import concourse.mybir as mybir
import concourse.tile as tile
from concourse import bacc
from concourse.bass_utils import run_bass_kernel_spmd

EPS = 1e-5
N_CORES = 8
N_TOTAL = 32768
F = 1024
NC_ROWS = N_TOTAL // N_CORES  # 4096 rows per core
NT = NC_ROWS // 128           # 32 row-tiles per core
NG2 = NC_ROWS // 256          # 16 super-tiles (fp8 DoubleRow, K=256)
P = 128
FH = F // 2                   # 512
FQ = F // 4                   # 256
KB = F // P                   # 8 column blocks of 128

F32 = mybir.dt.float32
F16 = mybir.dt.float16
F8 = mybir.dt.float8e4
DR = mybir.MatmulPerfMode.DoubleRow

GRAM_MODE = "fp8dr"  # "fp16" | "fp8dr"

# fp16 gram tiles (mf, nq): rows mf*128..+128, cols nq*256..+256, ordered
# by mf; pass A = mf 0..6 (16 tiles = 8 PSUM banks x 2 halves), B = mf 7.
GRAM_TILES = sorted([(mf, nq) for nq in range(4) for mf in range(2 * nq, KB)])
GRAM_A = [t for t in GRAM_TILES if t[0] < 7]
GRAM_B = [t for t in GRAM_TILES if t[0] == 7]

# fp8dr gram tiles (mf, nh): rows mf*128..+128, cols nh*512..+512
DR_A = [(mf, 0) for mf in range(KB)]       # left half, 8 banks
DR_B1 = [(4, 1), (5, 1)]                   # lower-right quarter in three
DR_B2 = [(6, 1)]                           # passes so copies stream and
DR_B3 = [(7, 1)]                           # the final tail is one tile


def build_phase1_fp16() -> bass.Bass:
    nc = bacc.Bacc(None, target_bir_lowering=False, debug=False)

    x_in = nc.dram_tensor("x", [NC_ROWS, F], F16, kind="ExternalInput")
    gram_out = nc.dram_tensor("gram", [F, F], F32, kind="ExternalOutput")
    colsum_out = nc.dram_tensor("colsum", [P, F], F32, kind="ExternalOutput")

    with tile.TileContext(nc) as tc:
        with (
            tc.tile_pool(name="xres", bufs=1) as xres,
            tc.tile_pool(name="work", bufs=1) as work,
            tc.tile_pool(name="gout", bufs=8) as gout,
            tc.tile_pool(name="psum", bufs=8, space="PSUM") as psum,
        ):
            xt = []
            for nt in range(NT):
                t = xres.tile([P, F], F16, tag=f"x{nt}")
                if nt == 0:
                    nc.sync.dma_start(out=t[:, 0:FH], in_=x_in[0:P, 0:FH])
                    nc.sync.dma_start(out=t[:, FH:F], in_=x_in[0:P, FH:F])
                else:
                    eng = nc.sync if nt % 2 == 0 else nc.scalar
                    eng.dma_start(out=t, in_=x_in[nt * P : (nt + 1) * P, :])
                xt.append(t)

            acc = []
            for j in range(4):
                a = work.tile([P, F], F32, tag=f"acc{j}")
                nc.vector.memset(a, 0.0)
                acc.append(a)
            for nt in range(NT):
                nc.vector.tensor_add(acc[nt % 4], acc[nt % 4], xt[nt])
            nc.vector.tensor_add(acc[0], acc[0], acc[1])
            nc.vector.tensor_add(acc[2], acc[2], acc[3])
            nc.vector.tensor_add(acc[0], acc[0], acc[2])
            nc.sync.dma_start(out=colsum_out[:, :], in_=acc[0])

            for pi, tiles in enumerate([GRAM_A, GRAM_B]):
                npair = (len(tiles) + 1) // 2
                ps = [
                    psum.tile([P, 2, FQ], F32, tag="g", name=f"g_{pi}_{i}")
                    for i in range(npair)
                ]
                for nt in range(NT):
                    for i, (mf, nq) in enumerate(tiles):
                        nc.tensor.matmul(
                            ps[i % npair][:, i // npair, :],
                            xt[nt][:, mf * P : (mf + 1) * P],
                            xt[nt][:, nq * FQ : (nq + 1) * FQ],
                            start=(nt == 0 and i < npair),
                            stop=(nt == NT - 1),
                        )
                for j in range(npair):
                    for h in range(2):
                        i = j + h * npair
                        if i >= len(tiles):
                            continue
                        mf, nq = tiles[i]
                        g_sb = gout.tile(
                            [P, FQ], F32, tag="gsb", name=f"gsb_{mf}_{nq}"
                        )
                        if h == 0:
                            nc.scalar.copy(g_sb, ps[j][:, h, :])
                        else:
                            nc.vector.tensor_copy(g_sb, ps[j][:, h, :])
                        eng = nc.sync if (j + h) % 2 == 0 else nc.scalar
                        eng.dma_start(
                            out=gram_out[
                                mf * P : (mf + 1) * P, nq * FQ : (nq + 1) * FQ
                            ],
                            in_=g_sb,
                        )

    nc.compile()
    return nc


def build_phase1_fp8dr() -> bass.Bass:
    """e4m3 DoubleRow gram: K=256 per matmul, [128,512] tiles; colsum via
    ones-stationary matmuls (keeps VectorE off the critical path)."""
    nc = bacc.Bacc(None, target_bir_lowering=False, debug=False)

    x_in = nc.dram_tensor("x", [NC_ROWS, F], F8, kind="ExternalInput")
    gram_out = nc.dram_tensor("gram", [F, F], F32, kind="ExternalOutput")
    colsum_out = nc.dram_tensor("colsum", [P, F], F32, kind="ExternalOutput")

    # super-tile g holds rows g*256..(g+1)*256 as [pi, po, f], row=g*256+po*128+pi
    x_r = x_in.rearrange("(g po p) f -> p g po f", p=P, po=2)

    with tile.TileContext(nc) as tc:
        with (
            tc.tile_pool(name="xres", bufs=1) as xres,
            tc.tile_pool(name="work", bufs=1) as work,
            tc.tile_pool(name="gout", bufs=8) as gout,
            tc.tile_pool(name="psum", bufs=8, space="PSUM") as psum,
        ):
            # short HAM warmup on a memset tile (no DMA dependency): ~3.4us
            # of PE busy ending as the first x chunk lands, so the DR
            # stream starts at full clock
            ww = work.tile([P, FH], F8)
            nc.vector.memset(ww, 0.125)
            wu = psum.tile([P, FH], F32, tag="g", name="wu")
            NWU = 8
            for i in range(NWU):
                nc.tensor.matmul(
                    wu,
                    ww[:, 0:P],
                    ww,
                    start=(i == 0),
                    stop=(i == NWU - 1),
                )
            wu_sb = gout.tile([P, FH], F32, tag="gsb", name="wu_sb")
            nc.scalar.copy(wu_sb[:, 0:P], wu[:, 0:P])

            # colsum on VectorE with two wide [P,2,F] accumulators (few,
            # big DVE ops -- per-instruction overhead dominates small adds);
            # hidden under the LDW-gated matmul stream
            acc = []
            for j in range(2):
                a = work.tile([P, 2, F], F32, tag=f"acc{j}")
                nc.vector.memset(a, 0.0)
                acc.append(a)
            csum = work.tile([P, F], F32)

            xs = []
            for g in range(NG2):
                t = xres.tile([P, 2, F], F8, tag=f"x{g}")
                if g == 0:
                    # first matmul touches only cols 0:512 of both halves
                    nc.sync.dma_start(out=t[:, :, 0:FH], in_=x_r[:, 0, :, 0:FH])
                    nc.sync.dma_start(out=t[:, :, FH:F], in_=x_r[:, 0, :, FH:F])
                else:
                    nc.sync.dma_start(out=t, in_=x_r[:, g])
                xs.append(t)
                nc.vector.tensor_add(acc[g % 2], acc[g % 2], t)
            nc.vector.tensor_add(acc[0], acc[0], acc[1])
            nc.vector.tensor_add(csum, acc[0][:, 0, :], acc[0][:, 1, :])
            nc.sync.dma_start(out=colsum_out[:, :], in_=csum)

            for pi, tiles in enumerate([DR_A, DR_B1, DR_B2, DR_B3]):
                ps = [
                    psum.tile([P, FH], F32, tag="g", name=f"g_{pi}_{i}")
                    for i in range(len(tiles))
                ]
                for g in range(NG2):
                    for i, (mf, nh) in enumerate(tiles):
                        nc.tensor.matmul(
                            ps[i],
                            xs[g][:, :, mf * P : (mf + 1) * P],
                            xs[g][:, :, nh * FH : (nh + 1) * FH],
                            start=(g == 0),
                            stop=(g == NG2 - 1),
                            perf_mode=DR,
                        )
                for i, (mf, nh) in enumerate(tiles):
                    g_sb = gout.tile([P, FH], F32, tag="gsb", name=f"gsb_{mf}_{nh}")
                    nc.scalar.copy(g_sb, ps[i])
                    # last passes write via scalar HWDGE so the gpsimd ring
                    # drains before the epilogue barrier
                    eng = nc.gpsimd if (pi == 0 and i % 2 == 0) else nc.scalar
                    eng.dma_start(
                        out=gram_out[
                            mf * P : (mf + 1) * P, nh * FH : (nh + 1) * FH
                        ],
                        in_=g_sb,
                    )

    nc.compile()
    return nc


# phase-2 W^T stationary blocks: (kb, mf) with kb <= mf (W lower tri)
WT_BLOCKS = [(kb, mf) for mf in range(KB) for kb in range(mf + 1)]
WT_IDX = {bm: i for i, bm in enumerate(WT_BLOCKS)}
NRG = NC_ROWS // FH  # 8 row groups of 512


def build_phase2() -> bass.Bass:
    """Per-core: yt [F, NC_ROWS] (fp16) = W @ Xq^T  (fp16 in, no bias)."""
    nc = bacc.Bacc(None, target_bir_lowering=False, debug=False)

    xt_in = nc.dram_tensor("xt", [F, NC_ROWS], F16, kind="ExternalInput")
    wtp_in = nc.dram_tensor(
        "wtp", [P, len(WT_BLOCKS), P], F16, kind="ExternalInput"
    )
    yt_out = nc.dram_tensor("yt", [F, NC_ROWS], F16, kind="ExternalOutput")

    xt_r = xt_in.rearrange("(kb p) n -> p kb n", p=P)  # [128, 8, NC_ROWS]

    with tile.TileContext(nc) as tc:
        with (
            tc.tile_pool(name="singles", bufs=1) as singles,
            tc.tile_pool(name="yout", bufs=52) as yout,
            tc.tile_pool(name="psum", bufs=8, space="PSUM") as psum,
        ):
            xtall = singles.tile([P, KB, NC_ROWS], F16)
            wtp = singles.tile([P, len(WT_BLOCKS), P], F16)
            ww = singles.tile([P, P], F16)

            # HAM warmup: matmuls on a memset tile start right after the
            # preamble (no DMA dependency), so the PE reaches full clock
            # before the real stream and the early DMA waits are absorbed
            nc.vector.memset(ww, 0.125)
            wu = psum.tile([P, FH], F32, tag="ps", name="wu")
            NWU = 56
            for i in range(NWU):
                nc.tensor.matmul(
                    wu[:, 0:P],
                    ww,
                    ww,
                    start=(i == 0),
                    stop=(i == NWU - 1),
                )
            wu_sb = yout.tile([P, FH], F16, tag="y", name="wu_sb")
            nc.vector.tensor_copy(wu_sb[:, 0:P], wu[:, 0:P])

            # all reads share the sync ring, which drains FIFO -- exact
            # need-order priority: wtp blocks for phase mf land just
            # before xt chunk kb=mf (both gate phase mf); early chunks
            # split so matmuls unblock sooner
            def wtp_load(mf):
                lo, hi = mf * (mf + 1) // 2, (mf + 1) * (mf + 2) // 2
                nc.sync.dma_start(out=wtp[:, lo:hi, :], in_=wtp_in[:, lo:hi, :])

            wtp_load(0)
            for c0, c1 in [(0, 512), (512, 1024), (1024, 2048), (2048, 4096)]:
                nc.sync.dma_start(out=xtall[:, 0, c0:c1], in_=xt_r[:, 0, c0:c1])
            for kb in (1, 2):
                wtp_load(kb)
                for h in range(2):
                    nc.sync.dma_start(
                        out=xtall[:, kb, h * 2048 : (h + 1) * 2048],
                        in_=xt_r[:, kb, h * 2048 : (h + 1) * 2048],
                    )
            for kb in range(3, KB):
                wtp_load(kb)
                nc.sync.dma_start(out=xtall[:, kb, :], in_=xt_r[:, kb, :])

            # triangular apply: output f-block mf accumulates over kb<=mf;
            # stationary W^T block reused across the 8 row-group matmuls.
            # Early phases' writes are deferred past mf=3's matmuls so the
            # input stream gets clean HBM bandwidth; final mf runs in two
            # rg-halves so its copies/writes stream instead of piling up.
            deferred = []

            def emit_write(y_sb, mf, rg):
                # final phase's writes go via scalar HWDGE so the gpsimd
                # ring drains before the epilogue barrier
                eng = (
                    nc.scalar
                    if (mf == KB - 1 or (mf + rg) % 2 == 0)
                    else nc.gpsimd
                )
                eng.dma_start(
                    out=yt_out[mf * P : (mf + 1) * P, rg * FH : (rg + 1) * FH],
                    in_=y_sb,
                )

            for mf in range(KB):
                ps = [
                    psum.tile([P, FH], F32, tag="ps", name=f"ps_{mf}_{rg}")
                    for rg in range(NRG)
                ]
                # the last mf runs as two rg-halves so the first half's
                # copies/writes stream during the second half's matmuls
                rg_groups = (
                    [range(0, 4), range(4, 7), range(7, NRG)]
                    if mf == KB - 1
                    else [range(NRG)]
                )
                for rgs in rg_groups:
                    for kb in range(mf + 1):
                        w_st = wtp[:, WT_IDX[(kb, mf)], :]
                        for rg in rgs:
                            nc.tensor.matmul(
                                ps[rg],
                                w_st,
                                xtall[:, kb, rg * FH : (rg + 1) * FH],
                                start=(kb == 0),
                                stop=(kb == mf),
                            )
                if mf == 5:
                    # guard: gpsimd waits for the last read chunk before
                    # issuing the deferred writes, keeping HBM bandwidth
                    # clean for the whole input stream
                    guard = yout.tile([P, FH], F16, tag="y", name="guard")
                    nc.gpsimd.tensor_copy(guard[:, 0:8], xtall[:, KB - 1, 4088:4096])
                    for y_sb, dmf, drg in deferred:
                        nc.gpsimd.dma_start(
                            out=yt_out[
                                dmf * P : (dmf + 1) * P,
                                drg * FH : (drg + 1) * FH,
                            ],
                            in_=y_sb,
                        )
                    deferred = []
                for rg in range(NRG):
                    y_sb = yout.tile([P, FH], F16, tag="y", name=f"y_{mf}_{rg}")
                    if rg % 2 == 0:
                        nc.vector.tensor_copy(y_sb, ps[rg])
                    else:
                        nc.scalar.copy(y_sb, ps[rg])
                    if mf <= 4:
                        deferred.append((y_sb, mf, rg))
                    else:
                        emit_write(y_sb, mf, rg)

    nc.compile()
    return nc


_programs: dict = {}


def _get_programs():
    if "p1" not in _programs:
        _programs["p1"] = (
            build_phase1_fp8dr() if GRAM_MODE == "fp8dr" else build_phase1_fp16()
        )
        _programs["p2"] = build_phase2()
    return _programs["p1"], _programs["p2"]


def kernel(X, running_mean, running_cov, beta, trace=False):
    X = np.asarray(X, dtype=np.float32)
    beta = np.asarray(beta, dtype=np.float32)
    assert X.shape == (N_TOTAL, F)

    p1, p2 = _get_programs()
    core_ids = list(range(N_CORES))

    p1_dt = ml_dtypes.float8_e4m3 if GRAM_MODE == "fp8dr" else np.float16
    shards1 = X.astype(p1_dt).reshape(N_CORES, NC_ROWS, F)

    tkw = {"trace_cores": core_ids} if trace else {}

    def _run(prog, in_maps):
        # transient NRT/device hiccups (incl. NRT_EXEC_UNIT_UNRECOVERABLE
        # wedges) have been observed; back off and retry
        import time as _time

        for attempt, pause in enumerate((2.0, 10.0, 30.0)):
            try:
                return run_bass_kernel_spmd(
                    prog, in_maps, core_ids, trace=trace, **tkw
                )
            except Exception:
                _time.sleep(pause)
        return run_bass_kernel_spmd(prog, in_maps, core_ids, trace=trace, **tkw)

    in1 = [{"x": np.ascontiguousarray(shards1[i])} for i in range(N_CORES)]
    r1 = _run(p1, in1)
    kernel.exec_ns_phase1 = r1.exec_time_ns

    gram = np.zeros((F, F), dtype=np.float64)
    colsum = np.zeros((F,), dtype=np.float64)
    for res in r1.results:
        gram += res["gram"].astype(np.float64)
        colsum += res["colsum"].astype(np.float64).sum(axis=0)
    # mirror the computed lower triangle onto the upper
    gram = np.tril(gram) + np.tril(gram, -1).T

    mean = colsum / N_TOTAL
    cov = gram / N_TOTAL - np.outer(mean, mean)
    a = cov + EPS * np.eye(F, dtype=np.float64)
    L = np.linalg.cholesky(a)
    w = np.linalg.solve(L, np.eye(F, dtype=np.float64))  # W = L^-1
    wt = w.T  # upper triangular [k, f]
    wtp = np.zeros((P, len(WT_BLOCKS), P), dtype=np.float16)
    for (kb, mf), i in WT_IDX.items():
        wtp[:, i, :] = wt[kb * P : (kb + 1) * P, mf * P : (mf + 1) * P].astype(
            np.float16
        )
    b = (beta.astype(np.float64) - w @ mean).astype(np.float32)

    xts = np.ascontiguousarray(
        X.astype(np.float16).reshape(N_CORES, NC_ROWS, F).transpose(0, 2, 1)
    )
    in2 = [{"xt": xts[i], "wtp": wtp} for i in range(N_CORES)]
    r2 = _run(p2, in2)
    kernel.exec_ns_phase2 = r2.exec_time_ns

    # host epilogue: bias + upcast + transpose back (O(N F))
    y = np.empty((N_TOTAL, F), dtype=np.float32)
    for i, res in enumerate(r2.results):
        y[i * NC_ROWS : (i + 1) * NC_ROWS, :] = (
            res["yt"].astype(np.float32) + b[:, None]
        ).T
    return y


kernel.exec_ns_phase1 = None
kernel.exec_ns_phase2 = None
